# revision 1
# baseline (speedup 1.0000x reference)
"""Trainium2 Bass kernel for a 4-layer bigram-LM dense transformer.

Full-model shapes: B=2, T=2048, E=256, H=8, L=4, V=32000.

Sharding over 8 NeuronCores (self-contained, hardcoded):
  - 2-way data parallel over batch: cores 0-3 handle batch 0, cores 4-7
    batch 1 (a "batch group" of 4 cores each).
  - Within a batch group, per-token work (LN / QKV / wo / FFN) is
    replicated; attention (the exp-heavy part) is sharded 2 heads/core
    and re-assembled with one 4-rank AllGather per layer.
  - lm_head is sharded 4-way over vocab columns within the group
    (8000 cols/core, padded to 8192), so the dominant 524MB logits
    write is split 8 ways.

Compute layout: activations live transposed [E, T] in SBUF so every
matmul contracts over the partition axis with zero transposes. All
matmul operands are float32r (FP22 mantissa truncation, full PE rate
at moving-dim >= 256). Softmax skips the max-subtraction (scores are
~1e-1 scale; exp cannot overflow) and row sums ride along in the
attention-output matmul via a ones column packed next to V, with
normalization folded into the PSUM->SBUF copy.
"""

import numpy as np

import concourse.bass as bass
import concourse.mybir as mybir
import concourse.tile as tile
from concourse import bacc
from concourse.bass_utils import run_bass_kernel_spmd

AF = mybir.ActivationFunctionType
ALU = mybir.AluOpType
FP32 = mybir.dt.float32
FP32R = mybir.dt.float32r

# model dims (full problem)
B, T, E, H, L, V = 2, 2048, 256, 8, 4, 32000
HD = E // H  # 32
EPS = 1e-5
NCORES = 8
GROUP = 4  # cores per batch group
HPC = H // GROUP  # heads per core (2)
VS = V // GROUP  # vocab shard per core (8000)
VSP = 8192  # padded vocab shard
TB = 512  # t-block (PSUM bank free dim)
SC = 128  # s-chunk (partition dim)
ET = E // 128  # embedding partition tiles (2)
FF = 4 * E  # 1024
UT = FF // 128  # ffn u-tiles (8)


def build_nc(t=T, layers=L, vsp=VSP, use_collective=True, ablate=()):
    """Build + compile the per-core Bass program (SPMD: same program, 8 cores)."""
    nt = t // TB      # t-blocks
    nsc = t // SC     # s-chunks
    ntc = t // 128    # t-chunks for lm head
    nvb = vsp // 512  # vocab blocks

    nc = bacc.Bacc("TRN2", num_devices=NCORES)

    # ---- DRAM parameters (per core) ----
    x0 = nc.declare_dram_parameter("x0", [E, t], FP32R, isOutput=False)
    wqkv = nc.declare_dram_parameter("wqkv", [layers, 128, ET, 6 * HD], FP32R, isOutput=False)
    wo_p = nc.declare_dram_parameter("wo", [layers, 128, ET, E], FP32R, isOutput=False)
    w1_p = nc.declare_dram_parameter("w1", [layers, 128, ET, FF], FP32R, isOutput=False)
    w2_p = nc.declare_dram_parameter("w2", [layers, 128, UT, E], FP32R, isOutput=False)
    vecs = nc.declare_dram_parameter("vecs", [layers, 128, 20], FP32, isOutput=False)
    fvec = nc.declare_dram_parameter("fvec", [128, 4], FP32, isOutput=False)
    whead = nc.declare_dram_parameter("whead", [128, ET, vsp], FP32R, isOutput=False)
    maskp = nc.declare_dram_parameter("mask", [SC, SC], FP32, isOutput=False)
    peye = nc.declare_dram_parameter("peye", [128, nt, nt], FP32R, isOutput=False)
    vtc = nc.declare_dram_parameter("vtc", [128, nsc, 2], FP32R, False)
    onesr = nc.declare_dram_parameter("onesr", [1, 128], FP32R, isOutput=False)
    selp = nc.declare_dram_parameter("selp", [nt, nt, 128], FP32R, isOutput=False)
    onesc = nc.declare_dram_parameter("onesc", [HD + 1, 128], FP32R, isOutput=False)
    logits = nc.declare_dram_parameter("logits", [t, vsp], FP32, isOutput=True)

    # internal DRAM bounce buffers for the per-layer AllGather
    cc_in = [nc.dram_tensor(f"cc_in{l}", [HPC * HD, t], FP32R) for l in range(layers)]
    cc_out = [nc.dram_tensor(f"cc_out{l}", [GROUP * HPC * HD, t], FP32R) for l in range(layers)]
    groups = [[0, 1, 2, 3], [4, 5, 6, 7]]

    from contextlib import ExitStack
    with tile.TileContext(nc) as tc:
        with ExitStack() as _ctx:
            persist = _ctx.enter_context(tc.tile_pool(name="persist", bufs=1))
            wpool2 = _ctx.enter_context(tc.tile_pool(name="wpool2", bufs=2))
            wpool1 = _ctx.enter_context(tc.tile_pool(name="wpool1", bufs=1))
            actp = _ctx.enter_context(tc.tile_pool(name="actp", bufs=1))
            xlnp = _ctx.enter_context(tc.tile_pool(name="xlnp", bufs=2))
            bigp = _ctx.enter_context(tc.tile_pool(name="bigp", bufs=3))
            expp = _ctx.enter_context(tc.tile_pool(name="expp", bufs=3))
            smallp = _ctx.enter_context(tc.tile_pool(name="smallp", bufs=2))
            tmpp = _ctx.enter_context(tc.tile_pool(name="tmpp", bufs=2))
            lgp = _ctx.enter_context(tc.tile_pool(name="lgp", bufs=6))
            whp = _ctx.enter_context(tc.tile_pool(name="whp", bufs=3))
            dpool = _ctx.enter_context(tc.tile_pool(name="dpool", bufs=2, space="DRAM"))
            ps_a = _ctx.enter_context(tc.tile_pool(name="ps_a", bufs=2, space="PSUM"))
            ps_o = _ctx.enter_context(tc.tile_pool(name="ps_o", bufs=1, space="PSUM"))
            ps_m = _ctx.enter_context(tc.tile_pool(name="ps_m", bufs=2, space="PSUM"))
            ps_s = _ctx.enter_context(tc.tile_pool(name="ps_s", bufs=1, space="PSUM"))
            # ---- persistent tiles ----
            xT = [persist.tile([128, t], FP32R, tag=f"xT{e}", name=f"xT{e}") for e in range(ET)]
            for e in range(ET):
                nc.sync.dma_start(out=xT[e], in_=x0[128 * e : 128 * (e + 1), :])
            mask = persist.tile([SC, SC], FP32, tag="mask")
            nc.sync.dma_start(out=mask, in_=maskp[:, :])
            fv = persist.tile([128, 4], FP32, tag="fvec")
            nc.sync.dma_start(out=fv, in_=fvec[:, :])
            # v tile: per chunk cols = [vA(32) | ones | vB(32) | ones] so the
            # 33-wide per-head lhsT computes o rows 0:32 plus a row-sum row 32
            vt = persist.tile([128, nsc, 2 * (HD + 1)], FP32R, tag="vt")
            nc.sync.dma_start(out=vt[:, :, HD : HD + 1], in_=vtc[:, :, 0:1])
            nc.sync.dma_start(out=vt[:, :, 2 * HD + 1 : 2 * HD + 2], in_=vtc[:, :, 1:2])
            eyeblk = persist.tile([128, nt, nt], FP32R, tag="eyeblk")
            nc.sync.dma_start(out=eyeblk, in_=peye[:, :, :])
            ones1c = persist.tile([1, 128], FP32R, tag="ones1c")
            nc.sync.dma_start(out=ones1c, in_=onesr[:, :])
            selt = persist.tile([nt, nt, 128], FP32R, tag="selt")
            nc.sync.dma_start(out=selt, in_=selp[:, :, :])
            ones33 = persist.tile([HD + 1, 128], FP32R, tag="ones33")
            nc.sync.dma_start(out=ones33, in_=onesc[:, :])
            # own heads' normalized attention out, pre-AllGather, [32, t] each
            oTp = [persist.tile([HD, t], FP32R, tag=f"oTp{h}", name=f"oTp{h}") for h in range(HPC)]
            epst = persist.tile([128, 1], FP32, tag="epst")
            nc.vector.memset(epst, EPS)

            def layernorm(src, g_ap_of, b_ap_of, out_tiles):
                if "ln" in ablate:
                    for e in range(ET):
                        nc.scalar.activation(
                            out=out_tiles[e][:, :], in_=src[e][:, :], func=AF.Identity,
                            bias=b_ap_of(e), scale=g_ap_of(e),
                        )
                    return
                """src: list of ET [128, t] fp32r tiles -> out_tiles fp32r.

                Per-token stats via ones-matmuls into PSUM rows {0,32,64,96}
                (one per t-block), then (x*s + m2)*g + b with s=rstd,
                m2=-mean*rstd broadcast along partitions.
                """
                sq = [
                    bigp.tile([128, t], FP32R, tag="big", name=f"sq{e}")
                    for e in range(ET)
                ]
                xs_ps = ps_s.tile([nt, TB], FP32, tag="stat_x")
                qs_ps = ps_s.tile([nt, TB], FP32, tag="stat_q")
                for tb in range(nt):
                    tbl = slice(TB * tb, TB * (tb + 1))
                    for e in range(ET):
                        nc.vector.tensor_tensor(
                            out=sq[e][:, tbl], in0=src[e][:, tbl],
                            in1=src[e][:, tbl], op=ALU.mult,
                        )
                    for e in range(ET):
                        nc.tensor.matmul(
                            xs_ps[:, :],
                            eyeblk[:, tb, :],
                            src[e][:, tbl],
                            start=(tb == 0 and e == 0),
                            stop=(tb == nt - 1 and e == ET - 1),
                        )
                    for e in range(ET):
                        nc.tensor.matmul(
                            qs_ps[:, :],
                            eyeblk[:, tb, :],
                            sq[e][:, tbl],
                            start=(tb == 0 and e == 0),
                            stop=(tb == nt - 1 and e == ET - 1),
                        )
                mean4 = smallp.tile([nt, TB], FP32, tag="mean4", name="mean4")
                msq4 = smallp.tile([nt, TB], FP32, tag="msq4", name="msq4")
                var4 = smallp.tile([nt, TB], FP32, tag="var4", name="var4")
                s4 = smallp.tile([nt, TB], FP32R, tag="s4", name="s4")
                xs_rows = xs_ps[:, :]
                qs_rows = qs_ps[:, :]
                nc.vector.tensor_scalar(mean4[:, :], xs_rows, 1.0 / E, None, ALU.mult)
                nc.vector.tensor_scalar(msq4[:, :], qs_rows, 1.0 / E, None, ALU.mult)
                nc.vector.tensor_tensor(
                    out=var4[:, :], in0=mean4[:, :], in1=mean4[:, :], op=ALU.mult
                )
                nc.vector.tensor_tensor(
                    out=var4[:, :], in0=msq4[:, :], in1=var4[:, :], op=ALU.subtract
                )
                nc.scalar.activation(out=var4[:, :], in_=var4[:, :], func=AF.Ln, bias=epst[0:nt, :])
                nc.scalar.activation(out=s4[:, :], in_=var4[:, :], func=AF.Exp, scale=-0.5)
                m24 = smallp.tile([nt, TB], FP32R, tag="msq4", name="m24")
                nc.vector.scalar_tensor_tensor(
                    out=m24[:, :], in0=mean4[:, :], scalar=-1.0, in1=s4[:, :],
                    op0=ALU.mult, op1=ALU.mult,
                )
                for tb in range(nt):
                    s_bc = ps_s.tile([128, TB], FP32, tag="stat_x", name="s_bc")
                    m_bc = ps_s.tile([128, TB], FP32, tag="stat_q", name="m_bc")
                    nc.tensor.matmul(
                        s_bc[:, :], selt[:, tb, :], s4[:, :],
                        start=True, stop=True,
                    )
                    nc.tensor.matmul(
                        m_bc[:, :], selt[:, tb, :], m24[:, :],
                        start=True, stop=True,
                    )
                    for e in range(ET):
                        tmp = tmpp.tile([128, TB], FP32, tag="lntmp")
                        nc.vector.tensor_tensor(
                            out=tmp,
                            in0=src[e][:, TB * tb : TB * (tb + 1)],
                            in1=s_bc[:, :], op=ALU.mult,
                        )
                        nc.vector.tensor_tensor(
                            out=tmp, in0=tmp, in1=m_bc[:, :], op=ALU.add,
                        )
                        nc.scalar.activation(
                            out=out_tiles[e][:, TB * tb : TB * (tb + 1)],
                            in_=tmp, func=AF.Identity,
                            bias=b_ap_of(e), scale=g_ap_of(e),
                        )

            # ================= layers =================
            for l in range(layers):
                wq_t = [wpool2.tile([128, 6 * HD], FP32R, tag=f"wqkv{e}", name=f"wqkv{e}") for e in range(ET)]
                wo_t = [wpool2.tile([128, E], FP32R, tag=f"wo{e}", name=f"wot{e}") for e in range(ET)]
                w1_t = [wpool1.tile([128, FF], FP32R, tag=f"w1{e}", name=f"w1t{e}") for e in range(ET)]
                w2_t = wpool1.tile([128, UT, E], FP32R, tag="w2")
                vec = wpool2.tile([128, 20], FP32, tag="vec")
                for e in range(ET):
                    nc.sync.dma_start(out=wq_t[e], in_=wqkv[l, :, e, :])
                    nc.sync.dma_start(out=wo_t[e], in_=wo_p[l, :, e, :])
                    nc.sync.dma_start(out=w1_t[e], in_=w1_p[l, :, e, :])
                nc.sync.dma_start(out=w2_t, in_=w2_p[l, :, :, :])
                nc.sync.dma_start(out=vec, in_=vecs[l, :, :])

                xln = [xlnp.tile([128, t], FP32R, tag=f"xln{e}", name=f"xln{e}") for e in range(ET)]
                layernorm(
                    xT,
                    g_ap_of=lambda e: vec[:, 0 + e : 1 + e],
                    b_ap_of=lambda e: vec[:, 2 + e : 3 + e],
                    out_tiles=xln,
                )

                qT = actp.tile([2 * HD, t], FP32R, tag="qT")
                kT = actp.tile([2 * HD, t], FP32R, tag="kT")
                for tb in range(nt):
                    tsl = slice(TB * tb, TB * (tb + 1))
                    qp = ps_m.tile([2 * HD, TB], FP32, tag="m")
                    for e in range(ET):
                        nc.tensor.matmul(
                            qp[:, :], wq_t[e][:, 0 : 2 * HD], xln[e][:, tsl],
                            start=(e == 0), stop=(e == ET - 1),
                        )
                    nc.any.tensor_copy(out=qT[:, tsl], in_=qp[:, :])
                    kp = ps_a.tile([2 * HD, TB], FP32, tag="att", name="kp")
                    for e in range(ET):
                        nc.tensor.matmul(
                            kp[:, :], wq_t[e][:, 2 * HD : 4 * HD], xln[e][:, tsl],
                            start=(e == 0), stop=(e == ET - 1),
                        )
                    nc.any.tensor_copy(out=kT[:, tsl], in_=kp[:, :])
                    for i in range(4 * tb, 4 * tb + 4):
                        vp = ps_a.tile([128, 2 * HD], FP32, tag="att", name="vp")
                        for e in range(ET):
                            nc.tensor.matmul(
                                vp[:, :],
                                xln[e][:, SC * i : SC * (i + 1)],
                                wq_t[e][:, 4 * HD : 6 * HD],
                                start=(e == 0), stop=(e == ET - 1),
                            )
                        nc.any.tensor_copy(out=vt[:, i, 0:HD], in_=vp[:, 0:HD])
                        nc.any.tensor_copy(
                            out=vt[:, i, HD + 1 : 2 * HD + 1], in_=vp[:, HD : 2 * HD]
                        )

                    # ---- attention for this t-block (qkv ready up to here) ----
                    if "attn" in ablate:
                        if tb == 0:
                            for h in range(HPC):
                                nc.vector.memset(oTp[h].bitcast(FP32), 1.0)
                        continue
                    op_ps = [
                        ps_o.tile([HD + 1, TB], FP32, tag=f"o{h}", name=f"op_ps{h}")
                        for h in range(HPC)
                    ]
                    nmax = 4 * tb + 4
                    for h in range(HPC):
                        rsl = slice(32 * h, 32 * (h + 1))

                        def emit_o(i, exh, d):
                            nc.tensor.matmul(
                                op_ps[h][:, d:TB],
                                vt[:, i, (HD + 1) * h : (HD + 1) * h + HD + 1],
                                exh[:, d:TB],
                                start=(i == 0), stop=(i == nmax - 1),
                            )

                        pend = None
                        for i in range(nmax):
                            d = max(0, SC * i - TB * tb)
                            psl = slice(d, TB)
                            tgl = slice(TB * tb + d, TB * (tb + 1))
                            at_ps = ps_a.tile([128, TB], FP32, tag="att", name="at_ps")
                            exh = expp.tile([128, TB], FP32R, tag=f"exp{h}", name="exh")
                            nc.tensor.matmul(
                                at_ps[:, psl],
                                kT[rsl, SC * i : SC * (i + 1)],
                                qT[rsl, tgl],
                                start=True, stop=True,
                                tile_position=(32 * h, 0),
                            )
                            nc.scalar.activation(
                                out=exh[:, psl], in_=at_ps[:, psl],
                                func=AF.Exp, scale=float(E) ** -0.5,
                            )
                            if i >= 4 * tb:  # diagonal chunk: mask upper triangle
                                nc.vector.tensor_tensor(
                                    out=exh[:, d : d + SC],
                                    in0=exh[:, d : d + SC],
                                    in1=mask[:, :], op=ALU.mult,
                                )
                            if pend is not None:
                                emit_o(*pend)
                            pend = (i, exh, d)
                        emit_o(*pend)
                    # normalize each head by its row-sum (psum row 32)
                    srow = [
                        smallp.tile([HD + 1, TB], FP32, tag=f"srow{h}", name=f"srow{h}")
                        for h in range(HPC)
                    ]
                    rd = dpool.tile([HPC, TB], FP32, tag="rd", name="rd")
                    for h in range(HPC):
                        nc.vector.reciprocal(
                            out=srow[h][HD : HD + 1, :],
                            in_=op_ps[h][HD : HD + 1, :],
                        )
                        nc.sync.dma_start(
                            out=rd[h : h + 1, :],
                            in_=srow[h][HD : HD + 1, :],
                        )
                    rec_bc = tmpp.tile([HD, TB], FP32, tag="rbc", name="rec_bc")
                    for h in range(HPC):
                        nc.gpsimd.dma_start(
                            out=rec_bc,
                            in_=rd[h : h + 1, :].partition_broadcast(HD),
                        )
                        nc.vector.tensor_tensor(
                            out=oTp[h][:, TB * tb : TB * (tb + 1)],
                            in0=op_ps[h][0:HD, :],
                            in1=rec_bc,
                            op=ALU.mult,
                        )

                # ---- AllGather heads across the 4-core batch group ----
                oT = [actp.tile([128, t], FP32R, tag=tg, name=f"oT_{tg}") for tg in ("qT", "kT")]
                if use_collective:
                    for h in range(HPC):
                        nc.sync.dma_start(
                            out=cc_in[l][HD * h : HD * (h + 1), :], in_=oTp[h][:, :]
                        )
                    nc.gpsimd.collective_compute(
                        "AllGather", ALU.bypass,
                        replica_groups=groups,
                        ins=[cc_in[l][:, :]], outs=[cc_out[l][:, :]],
                    )
                    for e in range(ET):
                        nc.sync.dma_start(
                            out=oT[e], in_=cc_out[l][128 * e : 128 * (e + 1), :]
                        )
                else:  # single-group debug path (no comm): own heads only
                    for e in range(ET):
                        nc.vector.memset(oT[e].bitcast(FP32), 0.0)
                    for h in range(HPC):
                        nc.vector.tensor_copy(
                            out=oT[0][HD * h : HD * (h + 1), :].bitcast(FP32),
                            in_=oTp[h][:, :].bitcast(FP32),
                        )

                # ---- wo projection + residual ----
                for tb in range(nt):
                    tsl = slice(TB * tb, TB * (tb + 1))
                    for eo in range(ET):
                        wp = ps_m.tile([128, TB], FP32, tag="m")
                        for e in range(ET):
                            nc.tensor.matmul(
                                wp[:, :],
                                wo_t[e][:, 128 * eo : 128 * (eo + 1)],
                                oT[e][:, tsl],
                                start=(e == 0), stop=(e == ET - 1),
                            )
                        nc.vector.scalar_tensor_tensor(
                            out=xT[eo][:, tsl], in0=wp[:, :],
                            scalar=vec[:, 8 + eo : 9 + eo], in1=xT[eo][:, tsl],
                            op0=ALU.add, op1=ALU.add,
                        )

                # ---- FFN ----
                xln2 = [xlnp.tile([128, t], FP32R, tag=f"xln{e}", name=f"xln{e}") for e in range(ET)]
                layernorm(
                    xT,
                    g_ap_of=lambda e: vec[:, 4 + e : 5 + e],
                    b_ap_of=lambda e: vec[:, 6 + e : 7 + e],
                    out_tiles=xln2,
                )
                for tb in range(nt):
                    if "ffn" in ablate:
                        break
                    tsl = slice(TB * tb, TB * (tb + 1))
                    ru_halves = []
                    for half in range(2):
                        ru = bigp.tile([128, UT // 2, TB], FP32R, tag="big", name="ru")
                        for uu in range(UT // 2):
                            ut = half * (UT // 2) + uu
                            up = ps_a.tile([128, TB], FP32, tag="att", name="up")
                            for e in range(ET):
                                nc.tensor.matmul(
                                    up[:, :],
                                    w1_t[e][:, 128 * ut : 128 * (ut + 1)],
                                    xln2[e][:, tsl],
                                    start=(e == 0), stop=(e == ET - 1),
                                )
                            nc.scalar.activation(
                                out=ru[:, uu, :], in_=up[:, :], func=AF.Relu,
                                bias=vec[:, 10 + ut : 11 + ut],
                            )
                        ru_halves.append(ru)
                    for eo in range(ET):
                        wp2 = ps_m.tile([128, TB], FP32, tag="m", name="wp2")
                        for ut in range(UT):
                            nc.tensor.matmul(
                                wp2[:, :],
                                w2_t[:, ut, 128 * eo : 128 * (eo + 1)],
                                ru_halves[ut // (UT // 2)][:, ut % (UT // 2), :],
                                start=(ut == 0), stop=(ut == UT - 1),
                            )
                        nc.vector.scalar_tensor_tensor(
                            out=xT[eo][:, tsl], in0=wp2[:, :],
                            scalar=vec[:, 18 + eo : 19 + eo], in1=xT[eo][:, tsl],
                            op0=ALU.add, op1=ALU.add,
                        )

            # ================= final LN + lm_head =================
            xf = [xlnp.tile([128, t], FP32R, tag=f"xln{e}", name=f"xln{e}") for e in range(ET)]
            layernorm(
                xT,
                g_ap_of=lambda e: fv[:, 0 + e : 1 + e],
                b_ap_of=lambda e: fv[:, 2 + e : 3 + e],
                out_tiles=xf,
            )
            for vb in range(nvb if "lm" not in ablate else 1):
                wh = whp.tile([128, ET, 512], FP32R, tag="wh")
                nc.sync.dma_start(out=wh, in_=whead[:, :, 512 * vb : 512 * (vb + 1)])
                for tcn in range(ntc):
                    lp = ps_m.tile([128, 512], FP32, tag="m")
                    for e in range(ET):
                        nc.tensor.matmul(
                            lp[:, :],
                            xf[e][:, 128 * tcn : 128 * (tcn + 1)],
                            wh[:, e, :],
                            start=(e == 0), stop=(e == ET - 1),
                        )
                    lg = lgp.tile([128, 512], FP32, tag="lg")
                    if (vb + tcn) % 2 == 0:
                        nc.vector.tensor_copy(out=lg, in_=lp[:, :])
                    else:
                        nc.scalar.copy(out=lg, in_=lp[:, :])
                    nc.sync.dma_start(
                        out=logits[128 * tcn : 128 * (tcn + 1), 512 * vb : 512 * (vb + 1)],
                        in_=lg,
                    )

    nc.compile()
    return nc


# ---------------- host-side prep / unshard ----------------

def prep_core_inputs(c, X, tok_emb, pos_emb, wq, wk, wv, wo, bo, w1, b1, w2, b2,
                     ln1_g, ln1_b, ln2_g, ln2_b, lnf_g, lnf_b, w_head, b_head,
                     t=T, layers=L, vsp=VSP):
    b = c // GROUP
    j = c % GROUP
    heads = [HPC * j + k for k in range(HPC)]

    f32 = np.float32
    Xb = np.asarray(X[b]).astype(np.int64)
    x0 = (np.asarray(tok_emb)[Xb] + np.asarray(pos_emb)[:t]).astype(f32).T  # [E, t]

    wq = np.asarray(wq); wk = np.asarray(wk); wv = np.asarray(wv)
    wqkv_h = np.empty((layers, 128, ET, 6 * HD), f32)
    wo_h = np.empty((layers, 128, ET, E), f32)
    w1_h = np.empty((layers, 128, ET, FF), f32)
    w2_h = np.empty((layers, 128, UT, E), f32)
    vecs_h = np.empty((layers, 128, 20), f32)
    for l in range(layers):
        qc = np.concatenate([wq[l, h] for h in heads], axis=1)  # [E, 64]
        kc = np.concatenate([wk[l, h] for h in heads], axis=1)
        vc = np.concatenate([wv[l, h] for h in heads], axis=1)
        qkv = np.concatenate([qc, kc, vc], axis=1)  # [E, 192]
        wqkv_h[l] = qkv.reshape(ET, 128, 6 * HD).transpose(1, 0, 2)
        wo_h[l] = np.asarray(wo[l]).reshape(ET, 128, E).transpose(1, 0, 2)
        w1_h[l] = np.asarray(w1[l]).reshape(ET, 128, FF).transpose(1, 0, 2)
        w2_h[l] = np.asarray(w2[l]).reshape(UT, 128, E).transpose(1, 0, 2)
        vv = np.concatenate([
            np.asarray(ln1_g[l]), np.asarray(ln1_b[l]),
            np.asarray(ln2_g[l]), np.asarray(ln2_b[l]),
            np.asarray(bo[l]), np.asarray(b1[l]), np.asarray(b2[l]),
        ]).astype(f32)  # 2560
        vecs_h[l] = vv.reshape(20, 128).T
    fvec_h = np.concatenate(
        [np.asarray(lnf_g), np.asarray(lnf_b)]
    ).astype(f32).reshape(4, 128).T

    w_head = np.asarray(w_head)
    vs = w_head.shape[1] // GROUP
    wh = np.zeros((E, vsp), f32)
    wh[:, :vs] = w_head[:, vs * j : vs * (j + 1)]
    whead_h = np.ascontiguousarray(wh.reshape(ET, 128, vsp).transpose(1, 0, 2))

    sp = np.arange(SC)[:, None]
    tp = np.arange(SC)[None, :]
    mask_h = (sp <= tp).astype(f32)

    nt = t // TB
    nsc = t // SC
    peye_h = np.zeros((128, nt, nt), f32)
    for tb in range(nt):
        peye_h[:, tb, tb] = 1.0
    vtc_h = np.ones((128, nsc, 2), f32)

    return {
        "x0": np.ascontiguousarray(x0),
        "wqkv": np.ascontiguousarray(wqkv_h),
        "wo": np.ascontiguousarray(wo_h),
        "w1": np.ascontiguousarray(w1_h),
        "w2": np.ascontiguousarray(w2_h),
        "vecs": np.ascontiguousarray(vecs_h),
        "fvec": np.ascontiguousarray(fvec_h),
        "whead": whead_h,
        "mask": mask_h,
        "peye": peye_h,
        "vtc": vtc_h,
        "onesr": np.ones((1, 128), f32),
        "onesc": np.ones((HD + 1, 128), f32),
        "selp": np.ascontiguousarray(
            np.broadcast_to(np.eye(nt, dtype=f32)[:, :, None], (nt, nt, 128))
        ),
    }


_NC_CACHE = {}


def _get_nc():
    if "nc" not in _NC_CACHE:
        _NC_CACHE["nc"] = build_nc()
    return _NC_CACHE["nc"]


def kernel(**inputs):
    nc = _get_nc()
    in_maps = [prep_core_inputs(c, **inputs) for c in range(NCORES)]
    res = run_bass_kernel_spmd(nc, in_maps, list(range(NCORES)))
    out = np.empty((B, T, V), np.float32)
    for c in range(NCORES):
        b, j = c // GROUP, c % GROUP
        out[b, :, VS * j : VS * (j + 1)] = res.results[c]["logits"][:, :VS]
    b_head = np.asarray(inputs["b_head"])
    if np.any(b_head):
        out += b_head[None, None, :]
    return out



# revision 29
# speedup vs baseline: 1.2685x; 1.2685x over previous
"""Trainium2 Bass kernel for a 4-layer bigram-LM dense transformer.

Full-model shapes: B=2, T=2048, E=256, H=8, L=4, V=32000.

Sharding over 8 NeuronCores (self-contained, hardcoded):
  - 2-way data parallel over batch: cores 0-3 handle batch 0, cores 4-7
    batch 1 (a "batch group" of 4 cores each).
  - Within a batch group, per-token work (LN / QKV / wo / FFN) is
    replicated; attention (the exp-heavy part) is sharded 2 heads/core
    and re-assembled with one 4-rank AllGather per layer (bf16 payload).
  - lm_head is sharded 4-way over vocab columns within the group
    (8000 cols/core, padded to 8192), so the dominant logits write is
    split 8 ways and emitted as fp16 (host converts back to fp32).

Compute layout: activations live transposed [E, T] in SBUF so every
matmul contracts over the partition axis with zero transposes. All
activations and weights are bf16 (PSUM accumulation stays fp32), which
doubles DVE elementwise throughput and halves HBM/collective traffic.
LN gains are folded into the consuming weights host-side and LN biases
become per-output constants (q/k copy biases, bo/b1/b_head), so the LN
apply is just two bf16 tensor_tensor ops. Softmax skips the
max-subtraction (scores are ~1e-1 scale; exp cannot overflow) and row
sums ride along in the attention-output matmul via a ones column packed
next to V; the 1/rowsum is fanned across partitions with a K=1
ones-matmul (no DRAM round-trip). The two heads' score matmuls are
interleaved so they occupy different 32-row PE strips (tile_position)
and run concurrently; the two attn@V matmuls are packed into one PSUM
bank at column offsets 0 and 64 and also run concurrently. Logits are
emitted fp16 in a [128, T/128, V-shard] layout so eight token-chunks
batch into each DMA (the HWDGE fixed cost per descriptor-set is ~625ns),
and the lm_head matmul/copy pipeline rotates across four PSUM banks.
"""

import numpy as np
import ml_dtypes

import concourse.bass as bass
import concourse.mybir as mybir
import concourse.tile as tile
from concourse import bacc
from concourse.bass_utils import run_bass_kernel_spmd

AF = mybir.ActivationFunctionType
ALU = mybir.AluOpType
FP32 = mybir.dt.float32
FP32R = mybir.dt.float32r
BF16 = mybir.dt.bfloat16
FP16 = mybir.dt.float16
NP_BF16 = ml_dtypes.bfloat16

# model dims (full problem)
B, T, E, H, L, V = 2, 2048, 256, 8, 4, 32000
HD = E // H  # 32
EPS = 1e-5
NCORES = 8
GROUP = 4  # cores per batch group
HPC = H // GROUP  # heads per core (2)
VS = V // GROUP  # vocab shard per core (8000)
VSP = 8192  # padded vocab shard
TB = 512  # t-block (PSUM bank free dim)
SC = 128  # s-chunk (partition dim)
ET = E // 128  # embedding partition tiles (2)
FF = 4 * E  # 1024
UT = FF // 128  # ffn u-tiles (8)


def build_nc(t=T, layers=L, vsp=VSP, use_collective=True, ablate=()):
    """Build + compile the per-core Bass program (SPMD: same program, 8 cores)."""
    nt = t // TB      # t-blocks
    nsc = t // SC     # s-chunks
    ntc = t // 128    # t-chunks for lm head
    nvb = vsp // 512  # vocab blocks

    nc = bacc.Bacc("TRN2", num_devices=NCORES)

    # ---- DRAM parameters (per core) ----
    x0 = nc.declare_dram_parameter("x0", [E, t], BF16, isOutput=False)
    wqkv = nc.declare_dram_parameter("wqkv", [layers, 128, ET, 6 * HD], BF16, isOutput=False)
    wo_p = nc.declare_dram_parameter("wo", [layers, 128, ET, E], BF16, isOutput=False)
    w1_p = nc.declare_dram_parameter("w1", [layers, 128, ET, FF], BF16, isOutput=False)
    w2_p = nc.declare_dram_parameter("w2", [layers, 128, UT, E], BF16, isOutput=False)
    vecs = nc.declare_dram_parameter("vecs", [layers, 128, 20], FP32, isOutput=False)
    whead = nc.declare_dram_parameter("whead", [128, ET, vsp], BF16, isOutput=False)
    maskp = nc.declare_dram_parameter("mask", [SC, SC], BF16, isOutput=False)
    peye = nc.declare_dram_parameter("peye", [128, nt, nt], BF16, isOutput=False)
    vtc = nc.declare_dram_parameter("vtc", [128, nsc, 2], BF16, False)
    selp = nc.declare_dram_parameter("selp", [nt, nt, 128], FP32R, isOutput=False)
    logits = nc.declare_dram_parameter("logits", [128, t // 128, vsp], FP16, isOutput=True)

    # internal DRAM bounce buffers for the per-layer AllGather (bf16)
    cc_in = [nc.dram_tensor(f"cc_in{l}", [HPC * HD, t], BF16) for l in range(layers)]
    cc_out = [nc.dram_tensor(f"cc_out{l}", [GROUP * HPC * HD, t], BF16) for l in range(layers)]
    groups = [[0, 1, 2, 3], [4, 5, 6, 7]]

    from contextlib import ExitStack
    with tile.TileContext(nc) as tc:
        with ExitStack() as _ctx:
            persist = _ctx.enter_context(tc.tile_pool(name="persist", bufs=1))
            wpool2 = _ctx.enter_context(tc.tile_pool(name="wpool2", bufs=2))
            wpool1 = _ctx.enter_context(tc.tile_pool(name="wpool1", bufs=2))
            actp = _ctx.enter_context(tc.tile_pool(name="actp", bufs=1))
            xlnp = _ctx.enter_context(tc.tile_pool(name="xlnp", bufs=2))
            bigp = _ctx.enter_context(tc.tile_pool(name="bigp", bufs=3))
            expp = _ctx.enter_context(tc.tile_pool(name="expp", bufs=3))
            smallp = _ctx.enter_context(tc.tile_pool(name="smallp", bufs=3))
            tmpp = _ctx.enter_context(tc.tile_pool(name="tmpp", bufs=3))
            lgp = _ctx.enter_context(tc.tile_pool(name="lgp", bufs=3))
            whp = _ctx.enter_context(tc.tile_pool(name="whp", bufs=3))
            dpool = _ctx.enter_context(tc.tile_pool(name="dpool", bufs=2, space="DRAM"))
            ps_a = _ctx.enter_context(tc.tile_pool(name="ps_a", bufs=2, space="PSUM"))
            ps_o = _ctx.enter_context(tc.tile_pool(name="ps_o", bufs=2, space="PSUM"))
            ps_m = _ctx.enter_context(tc.tile_pool(name="ps_m", bufs=2, space="PSUM"))
            ps_s = _ctx.enter_context(tc.tile_pool(name="ps_s", bufs=1, space="PSUM"))
            # ---- persistent tiles ----
            xT = [persist.tile([128, t], BF16, tag=f"xT{e}", name=f"xT{e}") for e in range(ET)]
            for e in range(ET):
                nc.sync.dma_start(out=xT[e], in_=x0[128 * e : 128 * (e + 1), :])
            mask = persist.tile([SC, SC], BF16, tag="mask")
            nc.sync.dma_start(out=mask, in_=maskp[:, :])
            # v tile: per chunk cols = [vA(32) | ones | vB(32) | ones] so the
            # 33-wide per-head lhsT computes o rows 0:32 plus a row-sum row 32
            vt = persist.tile([128, nsc, 2, HD + 1], BF16, tag="vt")
            nc.sync.dma_start(out=vt[:, :, :, HD : HD + 1], in_=vtc[:, :, :])
            eyeblk = persist.tile([128, nt, nt], BF16, tag="eyeblk")
            nc.sync.dma_start(out=eyeblk, in_=peye[:, :, :])
            selt = persist.tile([nt, nt, 128], FP32R, tag="selt")
            nc.sync.dma_start(out=selt, in_=selp[:, :, :])
            # own heads' normalized attention out, pre-AllGather, [32, t] each
            oTp = [persist.tile([HD, t], BF16, tag=f"oTp{h}", name=f"oTp{h}") for h in range(HPC)]
            epst = persist.tile([128, 1], FP32, tag="epst")
            nc.vector.memset(epst, EPS)
            # ones rows at partitions 0 and 32: lhsT for the K=1 broadcast
            # matmul that fans the per-token 1/rowsum out to HD partitions
            ones33 = persist.tile([HD + 1, HD], FP32, tag="ones33")
            nc.vector.memset(ones33, 1.0)

            def layernorm(src, out_tiles):
                if "ln" in ablate:
                    for e in range(ET):
                        nc.scalar.activation(
                            out=out_tiles[e][:, :], in_=src[e][:, :], func=AF.Identity,
                        )
                    return
                """src: list of ET [128, t] bf16 tiles -> out_tiles bf16.

                Per-token stats via ones-matmuls into PSUM rows {0,32,64,96}
                (one per t-block), then x*s + m2 with s=rstd, m2=-mean*rstd
                broadcast along partitions. The LN gain/bias are folded into
                the consuming matmul's weights/biases host-side.
                """
                sq = [
                    bigp.tile([128, t], BF16, tag="big", name=f"sq{e}")
                    for e in range(ET)
                ]
                xs_ps = ps_s.tile([nt, TB], FP32, tag="stat_x")
                qs_ps = ps_s.tile([nt, TB], FP32, tag="stat_q")
                for tb in range(nt):
                    tbl = slice(TB * tb, TB * (tb + 1))
                    for e in range(ET):
                        nc.vector.tensor_tensor(
                            out=sq[e][:, tbl], in0=src[e][:, tbl],
                            in1=src[e][:, tbl], op=ALU.mult,
                        )
                    for e in range(ET):
                        nc.tensor.matmul(
                            xs_ps[:, :],
                            eyeblk[:, tb, :],
                            src[e][:, tbl],
                            start=(tb == 0 and e == 0),
                            stop=(tb == nt - 1 and e == ET - 1),
                        )
                    for e in range(ET):
                        nc.tensor.matmul(
                            qs_ps[:, :],
                            eyeblk[:, tb, :],
                            sq[e][:, tbl],
                            start=(tb == 0 and e == 0),
                            stop=(tb == nt - 1 and e == ET - 1),
                        )
                mean4 = smallp.tile([nt, TB], FP32, tag="mean4", name="mean4")
                msq4 = smallp.tile([nt, TB], FP32, tag="msq4", name="msq4")
                var4 = smallp.tile([nt, TB], FP32, tag="var4", name="var4")
                s4 = smallp.tile([nt, TB], FP32R, tag="s4", name="s4")
                xs_rows = xs_ps[:, :]
                qs_rows = qs_ps[:, :]
                nc.vector.tensor_scalar(mean4[:, :], xs_rows, 1.0 / E, None, ALU.mult)
                nc.vector.tensor_scalar(msq4[:, :], qs_rows, 1.0 / E, None, ALU.mult)
                nc.vector.tensor_tensor(
                    out=var4[:, :], in0=mean4[:, :], in1=mean4[:, :], op=ALU.mult
                )
                nc.vector.tensor_tensor(
                    out=var4[:, :], in0=msq4[:, :], in1=var4[:, :], op=ALU.subtract
                )
                nc.scalar.activation(out=var4[:, :], in_=var4[:, :], func=AF.Ln, bias=epst[0:nt, :])
                nc.scalar.activation(out=s4[:, :], in_=var4[:, :], func=AF.Exp, scale=-0.5)
                m24 = smallp.tile([nt, TB], FP32R, tag="msq4", name="m24")
                nc.vector.scalar_tensor_tensor(
                    out=m24[:, :], in0=mean4[:, :], scalar=-1.0, in1=s4[:, :],
                    op0=ALU.mult, op1=ALU.mult,
                )
                for tb in range(nt):
                    s_bc = ps_s.tile([128, TB], FP32, tag="stat_x", name="s_bc")
                    m_bc = ps_s.tile([128, TB], FP32, tag="stat_q", name="m_bc")
                    nc.tensor.matmul(
                        s_bc[:, :], selt[:, tb, :], s4[:, :],
                        start=True, stop=True,
                    )
                    nc.tensor.matmul(
                        m_bc[:, :], selt[:, tb, :], m24[:, :],
                        start=True, stop=True,
                    )
                    # stage broadcasts to bf16 SBUF once per t-block so the
                    # per-e apply runs in the DVE 2x bf16 mode
                    s_sb = tmpp.tile([128, TB], BF16, tag="lntmp", name="s_sb")
                    m_sb = tmpp.tile([128, TB], BF16, tag="rbc", name="m_sb")
                    nc.vector.tensor_copy(out=s_sb, in_=s_bc[:, :])
                    nc.scalar.copy(out=m_sb, in_=m_bc[:, :])
                    for e in range(ET):
                        tmp = bigp.tile([128, TB], BF16, tag="lnt2", name="lntmp2")
                        nc.vector.tensor_tensor(
                            out=tmp,
                            in0=src[e][:, TB * tb : TB * (tb + 1)],
                            in1=s_sb, op=ALU.mult,
                        )
                        nc.vector.tensor_tensor(
                            out=out_tiles[e][:, TB * tb : TB * (tb + 1)],
                            in0=tmp, in1=m_sb, op=ALU.add,
                        )

            # ================= layers =================
            for l in range(layers):
                wq_t = [wpool2.tile([128, 6 * HD], BF16, tag=f"wqkv{e}", name=f"wqkv{e}") for e in range(ET)]
                wo_t = [wpool2.tile([128, E], BF16, tag=f"wo{e}", name=f"wot{e}") for e in range(ET)]
                w1_t = [wpool1.tile([128, FF], BF16, tag=f"w1{e}", name=f"w1t{e}") for e in range(ET)]
                w2_t = wpool1.tile([128, UT, E], BF16, tag="w2")
                vec = wpool2.tile([128, 20], FP32, tag="vec")
                for e in range(ET):
                    nc.sync.dma_start(out=wq_t[e], in_=wqkv[l, :, e, :])
                    nc.sync.dma_start(out=wo_t[e], in_=wo_p[l, :, e, :])
                    nc.sync.dma_start(out=w1_t[e], in_=w1_p[l, :, e, :])
                nc.sync.dma_start(out=w2_t, in_=w2_p[l, :, :, :])
                nc.sync.dma_start(out=vec, in_=vecs[l, :, :])

                xln = [xlnp.tile([128, t], BF16, tag=f"xln{e}", name=f"xln{e}") for e in range(ET)]
                layernorm(xT, out_tiles=xln)

                qT = actp.tile([2 * HD, t], BF16, tag="qT")
                kT = actp.tile([2 * HD, t], BF16, tag="kT")
                for tb in range(nt):
                    tsl = slice(TB * tb, TB * (tb + 1))
                    qp = ps_m.tile([2 * HD, TB], FP32, tag="m")
                    for e in range(ET):
                        nc.tensor.matmul(
                            qp[:, :], wq_t[e][:, 0 : 2 * HD], xln[e][:, tsl],
                            start=(e == 0), stop=(e == ET - 1),
                        )
                    # +cq: the ln1_b contribution to q, folded host-side
                    nc.vector.tensor_scalar(
                        qT[:, tsl], qp[:, :], vec[0 : 2 * HD, 0:1], None, ALU.add
                    )
                    kp = ps_a.tile([2 * HD, TB], FP32, tag="att", name="kp")
                    for e in range(ET):
                        nc.tensor.matmul(
                            kp[:, :], wq_t[e][:, 2 * HD : 4 * HD], xln[e][:, tsl],
                            start=(e == 0), stop=(e == ET - 1),
                        )
                    nc.scalar.activation(
                        out=kT[:, tsl], in_=kp[:, :], func=AF.Identity,
                        bias=vec[0 : 2 * HD, 1:2],
                    )
                    for i in range(4 * tb, 4 * tb + 4):
                        vp = ps_a.tile([128, 2, HD], FP32, tag="att", name="vp")
                        for e in range(ET):
                            nc.tensor.matmul(
                                vp[:, :, :],
                                xln[e][:, SC * i : SC * (i + 1)],
                                wq_t[e][:, 4 * HD : 6 * HD],
                                start=(e == 0), stop=(e == ET - 1),
                            )
                        nc.any.tensor_copy(out=vt[:, i, :, 0:HD], in_=vp[:, :, :])

                    # ---- attention for this t-block (qkv ready up to here) ----
                    if "attn" in ablate:
                        if tb == 0:
                            for h in range(HPC):
                                nc.vector.memset(oTp[h].bitcast(FP16), 1.0)
                        continue
                    # both heads' o (+row-sum) packed in one PSUM bank:
                    # head h occupies partitions [64h, 64h+33)
                    op_ps = ps_o.tile([128, TB], FP32, tag="o", name="op_ps")
                    nmax = 4 * tb + 4

                    def emit_o(i, h, exh, d):
                        nc.tensor.matmul(
                            op_ps[64 * h : 64 * h + HD + 1, d:TB],
                            vt[:, i, h, :],
                            exh[:, d:TB],
                            start=(i == 0), stop=(i == nmax - 1),
                            tile_position=(0, 64 * h),
                        )

                    pend = ()
                    for i in range(nmax):
                        d = max(0, SC * i - TB * tb)
                        psl = slice(d, TB)
                        tgl = slice(TB * tb + d, TB * (tb + 1))
                        cur = []
                        for h in range(HPC):
                            rsl = slice(32 * h, 32 * (h + 1))
                            at_ps = ps_a.tile([128, TB], FP32, tag="att", name=f"at_ps{h}")
                            exh = expp.tile([128, TB], BF16, tag=f"exp{h}", name="exh")
                            nc.tensor.matmul(
                                at_ps[:, psl],
                                kT[rsl, SC * i : SC * (i + 1)],
                                qT[rsl, tgl],
                                start=True, stop=True,
                                tile_position=(32 * h, 0),
                            )
                            nc.scalar.activation(
                                out=exh[:, psl], in_=at_ps[:, psl],
                                func=AF.Exp, scale=float(E) ** -0.5,
                            )
                            if i >= 4 * tb:  # diagonal chunk: mask upper triangle
                                nc.vector.tensor_tensor(
                                    out=exh[:, d : d + SC],
                                    in0=exh[:, d : d + SC],
                                    in1=mask[:, :], op=ALU.mult,
                                )
                            cur.append((i, h, exh, d))
                        for ent in pend:
                            emit_o(*ent)
                        pend = cur
                    for ent in pend:
                        emit_o(*ent)
                    # normalize each head by its row-sum (psum row 64h+32):
                    # reciprocal -> K=1 ones-matmul broadcast across HD
                    # partitions (PSUM) -> multiply
                    sr = smallp.tile([HD + 1, TB], FP32, tag="srow", name="sr")
                    for h in range(HPC):
                        # DVE outputs must start at a 32-aligned partition
                        nc.vector.reciprocal(
                            out=sr[HD * h : HD * h + 1, :],
                            in_=op_ps[64 * h + HD : 64 * h + HD + 1, :],
                        )
                        rec_ps = ps_s.tile(
                            [HD, TB], FP32,
                            tag=("stat_x" if h == 0 else "stat_q"), name="rec_ps",
                        )
                        nc.tensor.matmul(
                            rec_ps[:, :],
                            ones33[HD * h : HD * h + 1, :],
                            sr[HD * h : HD * h + 1, :],
                            start=True, stop=True,
                        )
                        rec_sb = tmpp.tile([HD, TB], BF16, tag="rbc", name="rec_sb")
                        nc.vector.tensor_copy(out=rec_sb, in_=rec_ps[:, :])
                        nc.vector.tensor_tensor(
                            out=oTp[h][:, TB * tb : TB * (tb + 1)],
                            in0=op_ps[64 * h : 64 * h + HD, :],
                            in1=rec_sb,
                            op=ALU.mult,
                        )

                # ---- AllGather heads across the 4-core batch group ----
                oT = [actp.tile([128, t], BF16, tag=tg, name=f"oT_{tg}") for tg in ("qT", "kT")]
                for h in range(HPC):
                    nc.sync.dma_start(
                        out=cc_in[l][HD * h : HD * (h + 1), :], in_=oTp[h][:, :]
                    )
                if use_collective:
                    nc.gpsimd.collective_compute(
                        "AllGather", ALU.bypass,
                        replica_groups=groups,
                        ins=[cc_in[l][:, :]], outs=[cc_out[l][:, :]],
                    )
                    for e in range(ET):
                        nc.sync.dma_start(
                            out=oT[e], in_=cc_out[l][128 * e : 128 * (e + 1), :]
                        )
                else:
                    # no-comm build (used for TimelineSim): same bounce DMAs as
                    # the real path so DMA time is modeled; cc_out carries
                    # no meaningful data (timing-only build)
                    for e in range(ET):
                        nc.sync.dma_start(
                            out=oT[e], in_=cc_out[l][128 * e : 128 * (e + 1), :]
                        )

                # ---- wo projection + residual ----
                for tb in range(nt):
                    tsl = slice(TB * tb, TB * (tb + 1))
                    for eo in range(ET):
                        wp = ps_m.tile([128, TB], FP32, tag="m")
                        for e in range(ET):
                            nc.tensor.matmul(
                                wp[:, :],
                                wo_t[e][:, 128 * eo : 128 * (eo + 1)],
                                oT[e][:, tsl],
                                start=(e == 0), stop=(e == ET - 1),
                            )
                        nc.vector.scalar_tensor_tensor(
                            out=xT[eo][:, tsl], in0=wp[:, :],
                            scalar=vec[:, 8 + eo : 9 + eo], in1=xT[eo][:, tsl],
                            op0=ALU.add, op1=ALU.add,
                        )

                # ---- FFN ----
                xln2 = [xlnp.tile([128, t], BF16, tag=f"xln{e}", name=f"xln{e}") for e in range(ET)]
                layernorm(xT, out_tiles=xln2)
                for tb in range(nt):
                    if "ffn" in ablate:
                        break
                    tsl = slice(TB * tb, TB * (tb + 1))
                    ru_halves = []
                    for half in range(2):
                        ru = bigp.tile([128, UT // 2, TB], BF16, tag="big", name="ru")
                        for uu in range(UT // 2):
                            ut = half * (UT // 2) + uu
                            up = ps_a.tile([128, TB], FP32, tag="att", name="up")
                            for e in range(ET):
                                nc.tensor.matmul(
                                    up[:, :],
                                    w1_t[e][:, 128 * ut : 128 * (ut + 1)],
                                    xln2[e][:, tsl],
                                    start=(e == 0), stop=(e == ET - 1),
                                )
                            nc.scalar.activation(
                                out=ru[:, uu, :], in_=up[:, :], func=AF.Relu,
                                bias=vec[:, 10 + ut : 11 + ut],
                            )
                        ru_halves.append(ru)
                    for eo in range(ET):
                        wp2 = ps_m.tile([128, TB], FP32, tag="m", name="wp2")
                        for ut in range(UT):
                            nc.tensor.matmul(
                                wp2[:, :],
                                w2_t[:, ut, 128 * eo : 128 * (eo + 1)],
                                ru_halves[ut // (UT // 2)][:, ut % (UT // 2), :],
                                start=(ut == 0), stop=(ut == UT - 1),
                            )
                        nc.vector.scalar_tensor_tensor(
                            out=xT[eo][:, tsl], in0=wp2[:, :],
                            scalar=vec[:, 18 + eo : 19 + eo], in1=xT[eo][:, tsl],
                            op0=ALU.add, op1=ALU.add,
                        )

            # ================= final LN + lm_head =================
            xf = [xlnp.tile([128, t], BF16, tag=f"xln{e}", name=f"xln{e}") for e in range(ET)]
            layernorm(xT, out_tiles=xf)
            GB = min(8, ntc)  # token-chunks batched per logits DMA
            for vb in range(nvb if "lm" not in ablate else 1):
                wh = whp.tile([128, ET, 512], BF16, tag="wh")
                nc.sync.dma_start(out=wh, in_=whead[:, :, 512 * vb : 512 * (vb + 1)])
                for g in range(ntc // GB):
                    lg = lgp.tile([128, GB, 512], FP16, tag="lg")
                    for k in range(GB):
                        tcn = GB * g + k
                        # rotate across both PSUM pools (4 banks) so the
                        # matmul/copy pipeline never waits on a bank
                        lpool = ps_m if k % 2 == 0 else ps_a
                        lp = lpool.tile([128, 512], FP32, tag="m" if k % 2 == 0 else "att")
                        for e in range(ET):
                            nc.tensor.matmul(
                                lp[:, :],
                                xf[e][:, 128 * tcn : 128 * (tcn + 1)],
                                wh[:, e, :],
                                start=(e == 0), stop=(e == ET - 1),
                            )
                        if (vb + tcn) % 2 == 0:
                            nc.vector.tensor_copy(out=lg[:, k, :], in_=lp[:, :])
                        else:
                            nc.scalar.copy(out=lg[:, k, :], in_=lp[:, :])
                    nc.sync.dma_start(
                        out=logits[:, GB * g : GB * (g + 1), 512 * vb : 512 * (vb + 1)],
                        in_=lg,
                    )

    nc.compile()
    return nc


# ---------------- host-side prep / unshard ----------------

def prep_core_inputs(c, X, tok_emb, pos_emb, wq, wk, wv, wo, bo, w1, b1, w2, b2,
                     ln1_g, ln1_b, ln2_g, ln2_b, lnf_g, lnf_b, w_head, b_head,
                     t=T, layers=L, vsp=VSP):
    b = c // GROUP
    j = c % GROUP
    heads = [HPC * j + k for k in range(HPC)]

    f32 = np.float32
    Xb = np.asarray(X[b]).astype(np.int64)
    x0 = (np.asarray(tok_emb)[Xb] + np.asarray(pos_emb)[:t]).astype(f32).T  # [E, t]

    wq = np.asarray(wq); wk = np.asarray(wk); wv = np.asarray(wv)
    wqkv_h = np.empty((layers, 128, ET, 6 * HD), f32)
    wo_h = np.empty((layers, 128, ET, E), f32)
    w1_h = np.empty((layers, 128, ET, FF), f32)
    w2_h = np.empty((layers, 128, UT, E), f32)
    vecs_h = np.zeros((layers, 128, 20), f32)
    for l in range(layers):
        # fold LN gains into the consuming weights and LN biases into
        # per-output-constant corrections (exact for affine LN):
        #   xln_true = xln_raw * g + b  =>  W^T xln_true = (gW)^T xln_raw + W^T b
        g1 = np.asarray(ln1_g[l]).astype(f32)[:, None]
        b1n = np.asarray(ln1_b[l]).astype(f32)
        g2 = np.asarray(ln2_g[l]).astype(f32)[:, None]
        b2n = np.asarray(ln2_b[l]).astype(f32)
        qc = np.concatenate([wq[l, h] for h in heads], axis=1)  # [E, 64]
        kc = np.concatenate([wk[l, h] for h in heads], axis=1)
        vc = np.concatenate([wv[l, h] for h in heads], axis=1)
        cq = qc.T @ b1n  # [64] q bias from ln1_b
        ck = kc.T @ b1n
        # v bias from ln1_b for ALL heads, folded through wo into bo
        cv_full = np.concatenate([wv[l, h].T @ b1n for h in range(H)])  # [E]
        bo_eff = np.asarray(bo[l]).astype(f32) + np.asarray(wo[l]).T @ cv_full
        b1_eff = np.asarray(b1[l]).astype(f32) + np.asarray(w1[l]).T @ b2n
        qkv = np.concatenate([qc, kc, vc], axis=1) * g1  # [E, 192]
        wqkv_h[l] = qkv.reshape(ET, 128, 6 * HD).transpose(1, 0, 2)
        wo_h[l] = np.asarray(wo[l]).reshape(ET, 128, E).transpose(1, 0, 2)
        w1_h[l] = (np.asarray(w1[l]) * g2).reshape(ET, 128, FF).transpose(1, 0, 2)
        w2_h[l] = np.asarray(w2[l]).reshape(UT, 128, E).transpose(1, 0, 2)
        vecs_h[l, 0:2 * HD, 0] = cq
        vecs_h[l, 0:2 * HD, 1] = ck
        vecs_h[l, :, 8:10] = bo_eff.reshape(2, 128).T
        vecs_h[l, :, 10:18] = b1_eff.reshape(8, 128).T
        vecs_h[l, :, 18:20] = np.asarray(b2[l]).astype(f32).reshape(2, 128).T

    w_head = np.asarray(w_head) * np.asarray(lnf_g).astype(f32)[:, None]
    vs = w_head.shape[1] // GROUP
    wh = np.zeros((E, vsp), f32)
    wh[:, :vs] = w_head[:, vs * j : vs * (j + 1)]
    whead_h = np.ascontiguousarray(wh.reshape(ET, 128, vsp).transpose(1, 0, 2))

    sp = np.arange(SC)[:, None]
    tp = np.arange(SC)[None, :]
    mask_h = (sp <= tp).astype(f32)

    nt = t // TB
    nsc = t // SC
    peye_h = np.zeros((128, nt, nt), f32)
    for tb in range(nt):
        peye_h[:, tb, tb] = 1.0
    vtc_h = np.ones((128, nsc, 2), f32)

    bf = NP_BF16
    return {
        "x0": np.ascontiguousarray(x0).astype(bf),
        "wqkv": np.ascontiguousarray(wqkv_h).astype(bf),
        "wo": np.ascontiguousarray(wo_h).astype(bf),
        "w1": np.ascontiguousarray(w1_h).astype(bf),
        "w2": np.ascontiguousarray(w2_h).astype(bf),
        "vecs": np.ascontiguousarray(vecs_h),
        "whead": whead_h.astype(bf),
        "mask": mask_h.astype(bf),
        "peye": peye_h.astype(bf),
        "vtc": vtc_h.astype(bf),
        "selp": np.ascontiguousarray(
            np.broadcast_to(np.eye(nt, dtype=f32)[:, :, None], (nt, nt, 128))
        ),
    }


_NC_CACHE = {}


def _get_nc():
    if "nc" not in _NC_CACHE:
        _NC_CACHE["nc"] = build_nc()
    return _NC_CACHE["nc"]


def kernel(**inputs):
    nc = _get_nc()
    in_maps = [prep_core_inputs(c, **inputs) for c in range(NCORES)]
    res = run_bass_kernel_spmd(nc, in_maps, list(range(NCORES)))
    out = np.empty((B, T, V), np.float32)
    for c in range(NCORES):
        b, j = c // GROUP, c % GROUP
        lg = res.results[c]["logits"]  # [128, T//128, VSP]
        lg = lg.transpose(1, 0, 2).reshape(T, VSP)
        out[b, :, VS * j : VS * (j + 1)] = lg[:, :VS].astype(np.float32)
    # b_head plus the final-LN bias folded through w_head (host-side)
    bh_eff = np.asarray(inputs["b_head"]).astype(np.float32) + (
        np.asarray(inputs["w_head"]).astype(np.float32).T
        @ np.asarray(inputs["lnf_b"]).astype(np.float32)
    )
    if np.any(bh_eff):
        out += bh_eff[None, None, :]
    return out


# revision 39
# speedup vs baseline: 1.3148x; 1.0365x over previous
"""Trainium2 Bass kernel for a 4-layer bigram-LM dense transformer.

Full-model shapes: B=2, T=2048, E=256, H=8, L=4, V=32000.

Sharding over 8 NeuronCores (self-contained, hardcoded):
  - 2-way data parallel over batch: cores 0-3 handle batch 0, cores 4-7
    batch 1 (a "batch group" of 4 cores each).
  - Within a batch group, per-token work (LN / QKV / wo / FFN) is
    replicated; attention (the exp-heavy part) is sharded 2 heads/core
    and re-assembled with one 4-rank AllGather per layer (bf16 payload).
  - lm_head is sharded 4-way over vocab columns within the group
    (8000 cols/core, padded to 8192), so the dominant logits write is
    split 8 ways and emitted as fp16 (host converts back to fp32).

Compute layout: activations live transposed [E, T] in SBUF so every
matmul contracts over the partition axis with zero transposes. All
activations and weights are bf16 (PSUM accumulation stays fp32), which
doubles DVE elementwise throughput and halves HBM/collective traffic.
LN gains are folded into the consuming weights host-side and LN biases
become per-output constants (q/k copy biases, bo/b1/b_head), so the LN
apply is just two bf16 tensor_tensor ops. Softmax skips the
max-subtraction (scores are ~1e-1 scale; exp cannot overflow) and row
sums ride along in the attention-output matmul via a ones column packed
next to V; the 1/rowsum is fanned across partitions with a K=1
ones-matmul (no DRAM round-trip). The two heads' score matmuls are
interleaved so they occupy different 32-row PE strips (tile_position)
and run concurrently; the two attn@V matmuls are packed into one PSUM
bank at column offsets 0 and 64 and also run concurrently. Logits are
emitted fp16 in a [128, T/128, V-shard] layout so eight token-chunks
batch into each DMA (the HWDGE fixed cost per descriptor-set is ~625ns),
and the lm_head matmul/copy pipeline rotates across four PSUM banks.
"""

import numpy as np
import ml_dtypes

import concourse.bass as bass
import concourse.mybir as mybir
import concourse.tile as tile
from concourse import bacc
from concourse.bass_utils import run_bass_kernel_spmd

AF = mybir.ActivationFunctionType
ALU = mybir.AluOpType
FP32 = mybir.dt.float32
FP32R = mybir.dt.float32r
BF16 = mybir.dt.bfloat16
FP16 = mybir.dt.float16
NP_BF16 = ml_dtypes.bfloat16

# model dims (full problem)
B, T, E, H, L, V = 2, 2048, 256, 8, 4, 32000
HD = E // H  # 32
EPS = 1e-5
NCORES = 8
GROUP = 4  # cores per batch group
HPC = H // GROUP  # heads per core (2)
VS = V // GROUP  # vocab shard per core (8000)
VSP = 8192  # padded vocab shard
TB = 512  # t-block (PSUM bank free dim)
SC = 128  # s-chunk (partition dim)
ET = E // 128  # embedding partition tiles (2)
FF = 4 * E  # 1024
UT = FF // 128  # ffn u-tiles (8)


def build_nc(t=T, layers=L, vsp=VSP, use_collective=True, ablate=()):
    """Build + compile the per-core Bass program (SPMD: same program, 8 cores)."""
    nt = t // TB      # t-blocks
    nsc = t // SC     # s-chunks
    ntc = t // 128    # t-chunks for lm head
    nvb = vsp // 512  # vocab blocks

    nc = bacc.Bacc("TRN2", num_devices=NCORES)

    # ---- DRAM parameters (per core) ----
    x0 = nc.declare_dram_parameter("x0", [E, t], BF16, isOutput=False)
    wqkv = nc.declare_dram_parameter("wqkv", [layers, 128, ET, 6 * HD], BF16, isOutput=False)
    wo_p = nc.declare_dram_parameter("wo", [layers, 128, ET, E], BF16, isOutput=False)
    w1_p = nc.declare_dram_parameter("w1", [layers, 128, ET, FF], BF16, isOutput=False)
    w2_p = nc.declare_dram_parameter("w2", [layers, 128, UT, E], BF16, isOutput=False)
    vecs = nc.declare_dram_parameter("vecs", [layers, 128, 20], FP32, isOutput=False)
    whead = nc.declare_dram_parameter("whead", [128, ET, vsp], BF16, isOutput=False)
    maskp = nc.declare_dram_parameter("mask", [SC, SC], BF16, isOutput=False)
    peye = nc.declare_dram_parameter("peye", [128, nt, nt], BF16, isOutput=False)
    vtc = nc.declare_dram_parameter("vtc", [128, nsc, 2], BF16, False)
    selp = nc.declare_dram_parameter("selp", [nt, nt, 128], FP32R, isOutput=False)
    logits = nc.declare_dram_parameter("logits", [128, t // 128, vsp], FP16, isOutput=True)

    # internal DRAM bounce buffers for the per-layer AllGather (bf16)
    cc_in = [nc.dram_tensor(f"cc_in{l}", [HPC * HD, t], BF16) for l in range(layers)]
    cc_out = [nc.dram_tensor(f"cc_out{l}", [GROUP * HPC * HD, t], BF16) for l in range(layers)]
    groups = [[0, 1, 2, 3], [4, 5, 6, 7]]

    from contextlib import ExitStack
    with tile.TileContext(nc) as tc:
        with ExitStack() as _ctx:
            persist = _ctx.enter_context(tc.tile_pool(name="persist", bufs=1))
            wpool2 = _ctx.enter_context(tc.tile_pool(name="wpool2", bufs=2))
            wpool1 = _ctx.enter_context(tc.tile_pool(name="wpool1", bufs=2))
            actp = _ctx.enter_context(tc.tile_pool(name="actp", bufs=1))
            xlnp = _ctx.enter_context(tc.tile_pool(name="xlnp", bufs=2))
            bigp = _ctx.enter_context(tc.tile_pool(name="bigp", bufs=3))
            expp = _ctx.enter_context(tc.tile_pool(name="expp", bufs=3))
            smallp = _ctx.enter_context(tc.tile_pool(name="smallp", bufs=3))
            tmpp = _ctx.enter_context(tc.tile_pool(name="tmpp", bufs=3))
            lgp = _ctx.enter_context(tc.tile_pool(name="lgp", bufs=3))
            whp = _ctx.enter_context(tc.tile_pool(name="whp", bufs=3))
            dpool = _ctx.enter_context(tc.tile_pool(name="dpool", bufs=2, space="DRAM"))
            ps_a = _ctx.enter_context(tc.tile_pool(name="ps_a", bufs=2, space="PSUM"))
            ps_o = _ctx.enter_context(tc.tile_pool(name="ps_o", bufs=2, space="PSUM"))
            ps_m = _ctx.enter_context(tc.tile_pool(name="ps_m", bufs=2, space="PSUM"))
            ps_s = _ctx.enter_context(tc.tile_pool(name="ps_s", bufs=1, space="PSUM"))
            # ---- persistent tiles ----
            xT = [persist.tile([128, t], BF16, tag=f"xT{e}", name=f"xT{e}") for e in range(ET)]
            for e in range(ET):
                nc.sync.dma_start(out=xT[e], in_=x0[128 * e : 128 * (e + 1), :])
            mask = persist.tile([SC, SC], BF16, tag="mask")
            nc.sync.dma_start(out=mask, in_=maskp[:, :])
            # v tile: per chunk cols = [vA(32) | ones | vB(32) | ones] so the
            # 33-wide per-head lhsT computes o rows 0:32 plus a row-sum row 32
            vt = persist.tile([128, nsc, 2, HD + 1], BF16, tag="vt")
            nc.sync.dma_start(out=vt[:, :, :, HD : HD + 1], in_=vtc[:, :, :])
            eyeblk = persist.tile([128, nt, nt], BF16, tag="eyeblk")
            nc.sync.dma_start(out=eyeblk, in_=peye[:, :, :])
            selt = persist.tile([nt, nt, 128], FP32R, tag="selt")
            nc.sync.dma_start(out=selt, in_=selp[:, :, :])
            # own heads' normalized attention out, pre-AllGather, [32, t] each
            oTp = [persist.tile([HD, t], BF16, tag=f"oTp{h}", name=f"oTp{h}") for h in range(HPC)]
            epst = persist.tile([128, 1], FP32, tag="epst")
            nc.vector.memset(epst, EPS)
            # ones rows at partitions 0 and 32: lhsT for the K=1 broadcast
            # matmul that fans the per-token 1/rowsum out to HD partitions
            ones33 = persist.tile([HD + 1, HD], FP32, tag="ones33")
            nc.vector.memset(ones33, 1.0)

            def layernorm(src, out_tiles):
                if "ln" in ablate:
                    for e in range(ET):
                        nc.scalar.activation(
                            out=out_tiles[e][:, :], in_=src[e][:, :], func=AF.Identity,
                        )
                    return
                """src: list of ET [128, t] bf16 tiles -> out_tiles bf16.

                Per-token stats via ones-matmuls into PSUM rows {0,32,64,96}
                (one per t-block), then x*s + m2 with s=rstd, m2=-mean*rstd
                broadcast along partitions. The LN gain/bias are folded into
                the consuming matmul's weights/biases host-side.
                """
                sq = [
                    bigp.tile([128, t], BF16, tag="big", name=f"sq{e}")
                    for e in range(ET)
                ]
                xs_ps = ps_s.tile([nt, TB], FP32, tag="stat_x")
                qs_ps = ps_s.tile([nt, TB], FP32, tag="stat_q")
                for tb in range(nt):
                    tbl = slice(TB * tb, TB * (tb + 1))
                    # split x^2 across DVE and ACT (ACT idles in LN phase;
                    # Square shares a table set with exp/ln fillers)
                    nc.vector.tensor_tensor(
                        out=sq[0][:, tbl], in0=src[0][:, tbl],
                        in1=src[0][:, tbl], op=ALU.mult,
                    )
                    nc.scalar.activation(
                        out=sq[1][:, tbl], in_=src[1][:, tbl], func=AF.Square,
                    )
                    for e in range(ET):
                        nc.tensor.matmul(
                            xs_ps[:, :],
                            eyeblk[:, tb, :],
                            src[e][:, tbl],
                            start=(tb == 0 and e == 0),
                            stop=(tb == nt - 1 and e == ET - 1),
                        )
                    for e in range(ET):
                        nc.tensor.matmul(
                            qs_ps[:, :],
                            eyeblk[:, tb, :],
                            sq[e][:, tbl],
                            start=(tb == 0 and e == 0),
                            stop=(tb == nt - 1 and e == ET - 1),
                        )
                mean4 = smallp.tile([nt, TB], FP32, tag="mean4", name="mean4")
                msq4 = smallp.tile([nt, TB], FP32, tag="msq4", name="msq4")
                var4 = smallp.tile([nt, TB], FP32, tag="var4", name="var4")
                s4 = smallp.tile([nt, TB], FP32R, tag="s4", name="s4")
                xs_rows = xs_ps[:, :]
                qs_rows = qs_ps[:, :]
                nc.vector.tensor_scalar(mean4[:, :], xs_rows, 1.0 / E, None, ALU.mult)
                nc.vector.tensor_scalar(msq4[:, :], qs_rows, 1.0 / E, None, ALU.mult)
                nc.vector.tensor_tensor(
                    out=var4[:, :], in0=mean4[:, :], in1=mean4[:, :], op=ALU.mult
                )
                nc.vector.tensor_tensor(
                    out=var4[:, :], in0=msq4[:, :], in1=var4[:, :], op=ALU.subtract
                )
                nc.scalar.activation(out=var4[:, :], in_=var4[:, :], func=AF.Ln, bias=epst[0:nt, :])
                nc.scalar.activation(out=s4[:, :], in_=var4[:, :], func=AF.Exp, scale=-0.5)
                m24 = smallp.tile([nt, TB], FP32R, tag="msq4", name="m24")
                nc.vector.scalar_tensor_tensor(
                    out=m24[:, :], in0=mean4[:, :], scalar=-1.0, in1=s4[:, :],
                    op0=ALU.mult, op1=ALU.mult,
                )
                for tb in range(nt):
                    s_bc = ps_s.tile([128, TB], FP32, tag="stat_x", name="s_bc")
                    m_bc = ps_s.tile([128, TB], FP32, tag="stat_q", name="m_bc")
                    nc.tensor.matmul(
                        s_bc[:, :], selt[:, tb, :], s4[:, :],
                        start=True, stop=True,
                    )
                    nc.tensor.matmul(
                        m_bc[:, :], selt[:, tb, :], m24[:, :],
                        start=True, stop=True,
                    )
                    # stage broadcasts to bf16 SBUF once per t-block so the
                    # per-e apply runs in the DVE 2x bf16 mode
                    s_sb = tmpp.tile([128, TB], BF16, tag="lntmp", name="s_sb")
                    m_sb = tmpp.tile([128, TB], BF16, tag="rbc", name="m_sb")
                    nc.scalar.copy(out=s_sb, in_=s_bc[:, :])
                    nc.scalar.copy(out=m_sb, in_=m_bc[:, :])
                    for e in range(ET):
                        tmp = bigp.tile([128, TB], BF16, tag="lnt2", name="lntmp2")
                        nc.vector.tensor_tensor(
                            out=tmp,
                            in0=src[e][:, TB * tb : TB * (tb + 1)],
                            in1=s_sb, op=ALU.mult,
                        )
                        nc.vector.tensor_tensor(
                            out=out_tiles[e][:, TB * tb : TB * (tb + 1)],
                            in0=tmp, in1=m_sb, op=ALU.add,
                        )

            # ================= layers =================
            for l in range(layers):
                wq_t = [wpool2.tile([128, 6 * HD], BF16, tag=f"wqkv{e}", name=f"wqkv{e}") for e in range(ET)]
                wo_t = [wpool2.tile([128, E], BF16, tag=f"wo{e}", name=f"wot{e}") for e in range(ET)]
                w1_t = [wpool1.tile([128, FF], BF16, tag=f"w1{e}", name=f"w1t{e}") for e in range(ET)]
                w2_t = wpool1.tile([128, UT, E], BF16, tag="w2")
                vec = wpool2.tile([128, 20], FP32, tag="vec")
                for e in range(ET):
                    nc.sync.dma_start(out=wq_t[e], in_=wqkv[l, :, e, :])
                    nc.sync.dma_start(out=wo_t[e], in_=wo_p[l, :, e, :])
                    nc.sync.dma_start(out=w1_t[e], in_=w1_p[l, :, e, :])
                nc.sync.dma_start(out=w2_t, in_=w2_p[l, :, :, :])
                nc.sync.dma_start(out=vec, in_=vecs[l, :, :])

                xln = [xlnp.tile([128, t], BF16, tag=f"xln{e}", name=f"xln{e}") for e in range(ET)]
                layernorm(xT, out_tiles=xln)

                qT = actp.tile([2 * HD, t], BF16, tag="qT")
                kT = actp.tile([2 * HD, t], BF16, tag="kT")
                for tb in range(nt):
                    tsl = slice(TB * tb, TB * (tb + 1))
                    qp = ps_m.tile([2 * HD, TB], FP32, tag="m")
                    for e in range(ET):
                        nc.tensor.matmul(
                            qp[:, :], wq_t[e][:, 0 : 2 * HD], xln[e][:, tsl],
                            start=(e == 0), stop=(e == ET - 1),
                        )
                    # +cq: the ln1_b contribution to q, folded host-side
                    nc.vector.tensor_scalar(
                        qT[:, tsl], qp[:, :], vec[0 : 2 * HD, 0:1], None, ALU.add
                    )
                    kp = ps_m.tile([2 * HD, TB], FP32, tag="m", name="kp")
                    for e in range(ET):
                        nc.tensor.matmul(
                            kp[:, :], wq_t[e][:, 2 * HD : 4 * HD], xln[e][:, tsl],
                            start=(e == 0), stop=(e == ET - 1),
                        )
                    nc.scalar.activation(
                        out=kT[:, tsl], in_=kp[:, :], func=AF.Identity,
                        bias=vec[0 : 2 * HD, 1:2],
                    )
                    for i in range(4 * tb, 4 * tb + 4):
                        vp = ps_o.tile([128, 2, HD], FP32, tag="o", name="vp")
                        for e in range(ET):
                            nc.tensor.matmul(
                                vp[:, :, :],
                                xln[e][:, SC * i : SC * (i + 1)],
                                wq_t[e][:, 4 * HD : 6 * HD],
                                start=(e == 0), stop=(e == ET - 1),
                            )
                        nc.any.tensor_copy(out=vt[:, i, :, 0:HD], in_=vp[:, :, :])

                    # ---- attention for this t-block (qkv ready up to here) ----
                    if "attn" in ablate:
                        if tb == 0:
                            for h in range(HPC):
                                nc.vector.memset(oTp[h].bitcast(FP16), 1.0)
                        continue
                    # both heads' o (+row-sum) packed in one PSUM bank:
                    # head h occupies partitions [64h, 64h+33)
                    op_ps = ps_o.tile([128, TB], FP32, tag="o", name="op_ps")
                    nmax = 4 * tb + 4

                    def emit_o(i, h, exh, d):
                        nc.tensor.matmul(
                            op_ps[64 * h : 64 * h + HD + 1, d:TB],
                            vt[:, i, h, :],
                            exh[:, d:TB],
                            start=(i == 0), stop=(i == nmax - 1),
                            tile_position=(0, 64 * h),
                        )

                    pend = ()
                    for i in range(nmax):
                        d = max(0, SC * i - TB * tb)
                        psl = slice(d, TB)
                        tgl = slice(TB * tb + d, TB * (tb + 1))
                        cur = []
                        for h in range(HPC):
                            rsl = slice(32 * h, 32 * (h + 1))
                            at_ps = ps_a.tile([128, TB], FP32, tag="att", name=f"at_ps{h}")
                            exh = expp.tile([128, TB], BF16, tag=f"exp{h}", name="exh")
                            nc.tensor.matmul(
                                at_ps[:, psl],
                                kT[rsl, SC * i : SC * (i + 1)],
                                qT[rsl, tgl],
                                start=True, stop=True,
                                tile_position=(32 * h, 0),
                            )
                            nc.scalar.activation(
                                out=exh[:, psl], in_=at_ps[:, psl],
                                func=AF.Exp, scale=float(E) ** -0.5,
                            )
                            if i >= 4 * tb:  # diagonal chunk: mask upper triangle
                                nc.vector.tensor_tensor(
                                    out=exh[:, d : d + SC],
                                    in0=exh[:, d : d + SC],
                                    in1=mask[:, :], op=ALU.mult,
                                )
                            cur.append((i, h, exh, d))
                        for ent in pend:
                            emit_o(*ent)
                        pend = cur
                    for ent in pend:
                        emit_o(*ent)
                    # normalize each head by its row-sum (psum row 64h+32):
                    # reciprocal -> K=1 ones-matmul broadcast across HD
                    # partitions (PSUM) -> multiply
                    sr = smallp.tile([HD + 1, TB], FP32, tag="srow", name="sr")
                    for h in range(HPC):
                        # DVE outputs must start at a 32-aligned partition
                        nc.vector.reciprocal(
                            out=sr[HD * h : HD * h + 1, :],
                            in_=op_ps[64 * h + HD : 64 * h + HD + 1, :],
                        )
                        rec_ps = ps_s.tile(
                            [HD, TB], FP32,
                            tag=("stat_x" if h == 0 else "stat_q"), name="rec_ps",
                        )
                        nc.tensor.matmul(
                            rec_ps[:, :],
                            ones33[HD * h : HD * h + 1, :],
                            sr[HD * h : HD * h + 1, :],
                            start=True, stop=True,
                        )
                        rec_sb = tmpp.tile([HD, TB], BF16, tag="rbc", name="rec_sb")
                        nc.vector.tensor_copy(out=rec_sb, in_=rec_ps[:, :])
                        nc.vector.tensor_tensor(
                            out=oTp[h][:, TB * tb : TB * (tb + 1)],
                            in0=op_ps[64 * h : 64 * h + HD, :],
                            in1=rec_sb,
                            op=ALU.mult,
                        )
                        # stream this t-block's slice to the AllGather bounce
                        # buffer now, so the collective input is ready the
                        # moment the last block finishes
                        nc.sync.dma_start(
                            out=cc_in[l][HD * h : HD * (h + 1), TB * tb : TB * (tb + 1)],
                            in_=oTp[h][:, TB * tb : TB * (tb + 1)],
                        )

                # ---- AllGather heads across the 4-core batch group ----
                oT = [actp.tile([128, t], BF16, tag=tg, name=f"oT_{tg}") for tg in ("qT", "kT")]
                if use_collective:
                    nc.gpsimd.collective_compute(
                        "AllGather", ALU.bypass,
                        replica_groups=groups,
                        ins=[cc_in[l][:, :]], outs=[cc_out[l][:, :]],
                    )
                    for e in range(ET):
                        nc.sync.dma_start(
                            out=oT[e], in_=cc_out[l][128 * e : 128 * (e + 1), :]
                        )
                else:
                    # no-comm build (used for TimelineSim): same bounce DMAs as
                    # the real path so DMA time is modeled; cc_out carries
                    # no meaningful data (timing-only build)
                    for e in range(ET):
                        nc.sync.dma_start(
                            out=oT[e], in_=cc_out[l][128 * e : 128 * (e + 1), :]
                        )

                # ---- wo projection + residual ----
                for tb in range(nt):
                    tsl = slice(TB * tb, TB * (tb + 1))
                    for eo in range(ET):
                        wpool = ps_m if eo % 2 == 0 else ps_a
                        wp = wpool.tile([128, TB], FP32, tag="m" if eo % 2 == 0 else "att")
                        for e in range(ET):
                            nc.tensor.matmul(
                                wp[:, :],
                                wo_t[e][:, 128 * eo : 128 * (eo + 1)],
                                oT[e][:, tsl],
                                start=(e == 0), stop=(e == ET - 1),
                            )
                        nc.vector.scalar_tensor_tensor(
                            out=xT[eo][:, tsl], in0=wp[:, :],
                            scalar=vec[:, 8 + eo : 9 + eo], in1=xT[eo][:, tsl],
                            op0=ALU.add, op1=ALU.add,
                        )

                # ---- FFN ----
                xln2 = [xlnp.tile([128, t], BF16, tag=f"xln{e}", name=f"xln{e}") for e in range(ET)]
                layernorm(xT, out_tiles=xln2)
                for tb in range(nt):
                    if "ffn" in ablate:
                        break
                    tsl = slice(TB * tb, TB * (tb + 1))
                    ru_halves = []
                    for half in range(2):
                        ru = bigp.tile([128, UT // 2, TB], BF16, tag="big", name="ru")
                        for uu in range(UT // 2):
                            ut = half * (UT // 2) + uu
                            upool = ps_a if uu % 2 == 0 else ps_o
                            up = upool.tile([128, TB], FP32, tag="att" if uu % 2 == 0 else "o", name="up")
                            for e in range(ET):
                                nc.tensor.matmul(
                                    up[:, :],
                                    w1_t[e][:, 128 * ut : 128 * (ut + 1)],
                                    xln2[e][:, tsl],
                                    start=(e == 0), stop=(e == ET - 1),
                                )
                            nc.scalar.activation(
                                out=ru[:, uu, :], in_=up[:, :], func=AF.Relu,
                                bias=vec[:, 10 + ut : 11 + ut],
                            )
                        ru_halves.append(ru)
                    for eo in range(ET):
                        wp2 = ps_m.tile([128, TB], FP32, tag="m", name="wp2")
                        for ut in range(UT):
                            nc.tensor.matmul(
                                wp2[:, :],
                                w2_t[:, ut, 128 * eo : 128 * (eo + 1)],
                                ru_halves[ut // (UT // 2)][:, ut % (UT // 2), :],
                                start=(ut == 0), stop=(ut == UT - 1),
                            )
                        nc.vector.scalar_tensor_tensor(
                            out=xT[eo][:, tsl], in0=wp2[:, :],
                            scalar=vec[:, 18 + eo : 19 + eo], in1=xT[eo][:, tsl],
                            op0=ALU.add, op1=ALU.add,
                        )

            # ================= final LN + lm_head =================
            xf = [xlnp.tile([128, t], BF16, tag=f"xln{e}", name=f"xln{e}") for e in range(ET)]
            layernorm(xT, out_tiles=xf)
            GB = min(8, ntc)  # token-chunks batched per logits DMA
            for vb in range(nvb if "lm" not in ablate else 1):
                wh = whp.tile([128, ET, 512], BF16, tag="wh")
                nc.sync.dma_start(out=wh, in_=whead[:, :, 512 * vb : 512 * (vb + 1)])
                for g in range(ntc // GB):
                    lg = lgp.tile([128, GB, 512], FP16, tag="lg")
                    for k in range(GB):
                        tcn = GB * g + k
                        # rotate across both PSUM pools (4 banks) so the
                        # matmul/copy pipeline never waits on a bank
                        lpool = ps_m if k % 2 == 0 else ps_a
                        lp = lpool.tile([128, 512], FP32, tag="m" if k % 2 == 0 else "att")
                        for e in range(ET):
                            nc.tensor.matmul(
                                lp[:, :],
                                xf[e][:, 128 * tcn : 128 * (tcn + 1)],
                                wh[:, e, :],
                                start=(e == 0), stop=(e == ET - 1),
                            )
                        if (vb + tcn) % 2 == 0:
                            nc.vector.tensor_copy(out=lg[:, k, :], in_=lp[:, :])
                        else:
                            nc.scalar.copy(out=lg[:, k, :], in_=lp[:, :])
                    nc.sync.dma_start(
                        out=logits[:, GB * g : GB * (g + 1), 512 * vb : 512 * (vb + 1)],
                        in_=lg,
                    )

    nc.compile()
    return nc


# ---------------- host-side prep / unshard ----------------

def prep_core_inputs(c, X, tok_emb, pos_emb, wq, wk, wv, wo, bo, w1, b1, w2, b2,
                     ln1_g, ln1_b, ln2_g, ln2_b, lnf_g, lnf_b, w_head, b_head,
                     t=T, layers=L, vsp=VSP):
    b = c // GROUP
    j = c % GROUP
    heads = [HPC * j + k for k in range(HPC)]

    f32 = np.float32
    Xb = np.asarray(X[b]).astype(np.int64)
    x0 = (np.asarray(tok_emb)[Xb] + np.asarray(pos_emb)[:t]).astype(f32).T  # [E, t]

    wq = np.asarray(wq); wk = np.asarray(wk); wv = np.asarray(wv)
    wqkv_h = np.empty((layers, 128, ET, 6 * HD), f32)
    wo_h = np.empty((layers, 128, ET, E), f32)
    w1_h = np.empty((layers, 128, ET, FF), f32)
    w2_h = np.empty((layers, 128, UT, E), f32)
    vecs_h = np.zeros((layers, 128, 20), f32)
    for l in range(layers):
        # fold LN gains into the consuming weights and LN biases into
        # per-output-constant corrections (exact for affine LN):
        #   xln_true = xln_raw * g + b  =>  W^T xln_true = (gW)^T xln_raw + W^T b
        g1 = np.asarray(ln1_g[l]).astype(f32)[:, None]
        b1n = np.asarray(ln1_b[l]).astype(f32)
        g2 = np.asarray(ln2_g[l]).astype(f32)[:, None]
        b2n = np.asarray(ln2_b[l]).astype(f32)
        qc = np.concatenate([wq[l, h] for h in heads], axis=1)  # [E, 64]
        kc = np.concatenate([wk[l, h] for h in heads], axis=1)
        vc = np.concatenate([wv[l, h] for h in heads], axis=1)
        cq = qc.T @ b1n  # [64] q bias from ln1_b
        ck = kc.T @ b1n
        # v bias from ln1_b for ALL heads, folded through wo into bo
        cv_full = np.concatenate([wv[l, h].T @ b1n for h in range(H)])  # [E]
        bo_eff = np.asarray(bo[l]).astype(f32) + np.asarray(wo[l]).T @ cv_full
        b1_eff = np.asarray(b1[l]).astype(f32) + np.asarray(w1[l]).T @ b2n
        qkv = np.concatenate([qc, kc, vc], axis=1) * g1  # [E, 192]
        wqkv_h[l] = qkv.reshape(ET, 128, 6 * HD).transpose(1, 0, 2)
        wo_h[l] = np.asarray(wo[l]).reshape(ET, 128, E).transpose(1, 0, 2)
        w1_h[l] = (np.asarray(w1[l]) * g2).reshape(ET, 128, FF).transpose(1, 0, 2)
        w2_h[l] = np.asarray(w2[l]).reshape(UT, 128, E).transpose(1, 0, 2)
        vecs_h[l, 0:2 * HD, 0] = cq
        vecs_h[l, 0:2 * HD, 1] = ck
        vecs_h[l, :, 8:10] = bo_eff.reshape(2, 128).T
        vecs_h[l, :, 10:18] = b1_eff.reshape(8, 128).T
        vecs_h[l, :, 18:20] = np.asarray(b2[l]).astype(f32).reshape(2, 128).T

    w_head = np.asarray(w_head) * np.asarray(lnf_g).astype(f32)[:, None]
    vs = w_head.shape[1] // GROUP
    wh = np.zeros((E, vsp), f32)
    wh[:, :vs] = w_head[:, vs * j : vs * (j + 1)]
    whead_h = np.ascontiguousarray(wh.reshape(ET, 128, vsp).transpose(1, 0, 2))

    sp = np.arange(SC)[:, None]
    tp = np.arange(SC)[None, :]
    mask_h = (sp <= tp).astype(f32)

    nt = t // TB
    nsc = t // SC
    peye_h = np.zeros((128, nt, nt), f32)
    for tb in range(nt):
        peye_h[:, tb, tb] = 1.0
    vtc_h = np.ones((128, nsc, 2), f32)

    bf = NP_BF16
    return {
        "x0": np.ascontiguousarray(x0).astype(bf),
        "wqkv": np.ascontiguousarray(wqkv_h).astype(bf),
        "wo": np.ascontiguousarray(wo_h).astype(bf),
        "w1": np.ascontiguousarray(w1_h).astype(bf),
        "w2": np.ascontiguousarray(w2_h).astype(bf),
        "vecs": np.ascontiguousarray(vecs_h),
        "whead": whead_h.astype(bf),
        "mask": mask_h.astype(bf),
        "peye": peye_h.astype(bf),
        "vtc": vtc_h.astype(bf),
        "selp": np.ascontiguousarray(
            np.broadcast_to(np.eye(nt, dtype=f32)[:, :, None], (nt, nt, 128))
        ),
    }


_NC_CACHE = {}


def _get_nc():
    if "nc" not in _NC_CACHE:
        _NC_CACHE["nc"] = build_nc()
    return _NC_CACHE["nc"]


def kernel(**inputs):
    nc = _get_nc()
    in_maps = [prep_core_inputs(c, **inputs) for c in range(NCORES)]
    res = run_bass_kernel_spmd(nc, in_maps, list(range(NCORES)))
    out = np.empty((B, T, V), np.float32)
    for c in range(NCORES):
        b, j = c // GROUP, c % GROUP
        lg = res.results[c]["logits"]  # [128, T//128, VSP]
        lg = lg.transpose(1, 0, 2).reshape(T, VSP)
        out[b, :, VS * j : VS * (j + 1)] = lg[:, :VS].astype(np.float32)
    # b_head plus the final-LN bias folded through w_head (host-side)
    bh_eff = np.asarray(inputs["b_head"]).astype(np.float32) + (
        np.asarray(inputs["w_head"]).astype(np.float32).T
        @ np.asarray(inputs["lnf_b"]).astype(np.float32)
    )
    if np.any(bh_eff):
        out += bh_eff[None, None, :]
    return out


# revision 46
# speedup vs baseline: 1.3169x; 1.0016x over previous
"""Trainium2 Bass kernel for a 4-layer bigram-LM dense transformer.

Full-model shapes: B=2, T=2048, E=256, H=8, L=4, V=32000.

Sharding over 8 NeuronCores (self-contained, hardcoded):
  - 2-way data parallel over batch: cores 0-3 handle batch 0, cores 4-7
    batch 1 (a "batch group" of 4 cores each).
  - Within a batch group, per-token work (LN / QKV / wo / FFN) is
    replicated; attention (the exp-heavy part) is sharded 2 heads/core
    and re-assembled with one 4-rank AllGather per layer (bf16 payload).
  - lm_head is sharded 4-way over vocab columns within the group
    (8000 cols/core, padded to 8192), so the dominant logits write is
    split 8 ways and emitted as fp16 (host converts back to fp32).

Compute layout: activations live transposed [E, T] in SBUF so every
matmul contracts over the partition axis with zero transposes. All
activations and weights are bf16 (PSUM accumulation stays fp32), which
doubles DVE elementwise throughput and halves HBM/collective traffic.
LN gains are folded into the consuming weights host-side and LN biases
become per-output constants (q/k copy biases, bo/b1/b_head), so the LN
apply is just two bf16 tensor_tensor ops. Softmax skips the
max-subtraction (scores are ~1e-1 scale; exp cannot overflow) and row
sums ride along in the attention-output matmul via a ones column packed
next to V; the 1/rowsum is fanned across partitions with a K=1
ones-matmul (no DRAM round-trip). The two heads' score matmuls are
interleaved so they occupy different 32-row PE strips (tile_position)
and run concurrently; the two attn@V matmuls are packed into one PSUM
bank at column offsets 0 and 64 and also run concurrently. Logits are
emitted fp16 in a [128, T/128, V-shard] layout so eight token-chunks
batch into each DMA (the HWDGE fixed cost per descriptor-set is ~625ns),
and the lm_head matmul/copy pipeline rotates across four PSUM banks.
"""

import numpy as np
import ml_dtypes

import concourse.bass as bass
import concourse.mybir as mybir
import concourse.tile as tile
from concourse import bacc
from concourse.bass_utils import run_bass_kernel_spmd

AF = mybir.ActivationFunctionType
ALU = mybir.AluOpType
FP32 = mybir.dt.float32
FP32R = mybir.dt.float32r
BF16 = mybir.dt.bfloat16
FP16 = mybir.dt.float16
NP_BF16 = ml_dtypes.bfloat16

# model dims (full problem)
B, T, E, H, L, V = 2, 2048, 256, 8, 4, 32000
HD = E // H  # 32
EPS = 1e-5
NCORES = 8
GROUP = 4  # cores per batch group
HPC = H // GROUP  # heads per core (2)
VS = V // GROUP  # vocab shard per core (8000)
VSP = 8192  # padded vocab shard
TB = 512  # t-block (PSUM bank free dim)
SC = 128  # s-chunk (partition dim)
ET = E // 128  # embedding partition tiles (2)
FF = 4 * E  # 1024
UT = FF // 128  # ffn u-tiles (8)


def build_nc(t=T, layers=L, vsp=VSP, use_collective=True, ablate=()):
    """Build + compile the per-core Bass program (SPMD: same program, 8 cores)."""
    nt = t // TB      # t-blocks
    nsc = t // SC     # s-chunks
    ntc = t // 128    # t-chunks for lm head
    nvb = vsp // 512  # vocab blocks

    nc = bacc.Bacc("TRN2", num_devices=NCORES)

    # ---- DRAM parameters (per core) ----
    x0 = nc.declare_dram_parameter("x0", [E, t], BF16, isOutput=False)
    wqkv = nc.declare_dram_parameter("wqkv", [layers, 128, ET, 6 * HD], BF16, isOutput=False)
    wo_p = nc.declare_dram_parameter("wo", [layers, 128, ET, E], BF16, isOutput=False)
    w1_p = nc.declare_dram_parameter("w1", [layers, 128, ET, FF], BF16, isOutput=False)
    w2_p = nc.declare_dram_parameter("w2", [layers, 128, UT, E], BF16, isOutput=False)
    vecs = nc.declare_dram_parameter("vecs", [layers, 128, 20], FP32, isOutput=False)
    whead = nc.declare_dram_parameter("whead", [128, ET, vsp], BF16, isOutput=False)
    maskp = nc.declare_dram_parameter("mask", [SC, SC], BF16, isOutput=False)
    peye = nc.declare_dram_parameter("peye", [128, nt, nt], BF16, isOutput=False)
    vtc = nc.declare_dram_parameter("vtc", [128, nsc, 2], BF16, False)
    selp = nc.declare_dram_parameter("selp", [nt, nt, 128], FP32R, isOutput=False)
    logits = nc.declare_dram_parameter("logits", [128, t // 128, vsp], FP16, isOutput=True)

    # internal DRAM bounce buffers for the per-layer AllGather (bf16)
    cc_in = [nc.dram_tensor(f"cc_in{l}", [HPC * HD, t], BF16) for l in range(layers)]
    cc_out = [nc.dram_tensor(f"cc_out{l}", [GROUP * HPC * HD, t], BF16) for l in range(layers)]
    groups = [[0, 1, 2, 3], [4, 5, 6, 7]]

    from contextlib import ExitStack
    with tile.TileContext(nc) as tc:
        with ExitStack() as _ctx:
            persist = _ctx.enter_context(tc.tile_pool(name="persist", bufs=1))
            wpool2 = _ctx.enter_context(tc.tile_pool(name="wpool2", bufs=2))
            wpool1 = _ctx.enter_context(tc.tile_pool(name="wpool1", bufs=2))
            actp = _ctx.enter_context(tc.tile_pool(name="actp", bufs=1))
            xlnp = _ctx.enter_context(tc.tile_pool(name="xlnp", bufs=2))
            bigp = _ctx.enter_context(tc.tile_pool(name="bigp", bufs=3))
            expp = _ctx.enter_context(tc.tile_pool(name="expp", bufs=4))
            smallp = _ctx.enter_context(tc.tile_pool(name="smallp", bufs=3))
            tmpp = _ctx.enter_context(tc.tile_pool(name="tmpp", bufs=3))
            lgp = _ctx.enter_context(tc.tile_pool(name="lgp", bufs=3))
            whp = _ctx.enter_context(tc.tile_pool(name="whp", bufs=3))
            dpool = _ctx.enter_context(tc.tile_pool(name="dpool", bufs=2, space="DRAM"))
            ps_a = _ctx.enter_context(tc.tile_pool(name="ps_a", bufs=2, space="PSUM"))
            ps_o = _ctx.enter_context(tc.tile_pool(name="ps_o", bufs=2, space="PSUM"))
            ps_m = _ctx.enter_context(tc.tile_pool(name="ps_m", bufs=2, space="PSUM"))
            ps_s = _ctx.enter_context(tc.tile_pool(name="ps_s", bufs=1, space="PSUM"))
            # ---- persistent tiles ----
            xT = [persist.tile([128, t], BF16, tag=f"xT{e}", name=f"xT{e}") for e in range(ET)]
            for e in range(ET):
                nc.sync.dma_start(out=xT[e], in_=x0[128 * e : 128 * (e + 1), :])
            mask = persist.tile([SC, SC], BF16, tag="mask")
            nc.sync.dma_start(out=mask, in_=maskp[:, :])
            # v tile: per chunk cols = [vA(32) | ones | vB(32) | ones] so the
            # 33-wide per-head lhsT computes o rows 0:32 plus a row-sum row 32
            vt = persist.tile([128, nsc, 2, HD + 1], BF16, tag="vt")
            nc.sync.dma_start(out=vt[:, :, :, HD : HD + 1], in_=vtc[:, :, :])
            eyeblk = persist.tile([128, nt, nt], BF16, tag="eyeblk")
            nc.sync.dma_start(out=eyeblk, in_=peye[:, :, :])
            selt = persist.tile([nt, nt, 128], FP32R, tag="selt")
            nc.sync.dma_start(out=selt, in_=selp[:, :, :])
            # own heads' normalized attention out, pre-AllGather, [32, t] each
            oTp = [persist.tile([HD, t], BF16, tag=f"oTp{h}", name=f"oTp{h}") for h in range(HPC)]
            epst = persist.tile([128, 1], FP32, tag="epst")
            nc.vector.memset(epst, EPS)
            # ones rows at partitions 0 and 32: lhsT for the K=1 broadcast
            # matmul that fans the per-token 1/rowsum out to HD partitions
            ones33 = persist.tile([HD + 1, HD], FP32, tag="ones33")
            nc.vector.memset(ones33, 1.0)

            def layernorm(src, out_tiles):
                if "ln" in ablate:
                    for e in range(ET):
                        nc.scalar.activation(
                            out=out_tiles[e][:, :], in_=src[e][:, :], func=AF.Identity,
                        )
                    return
                """src: list of ET [128, t] bf16 tiles -> out_tiles bf16.

                Per-token stats via ones-matmuls into PSUM rows {0,32,64,96}
                (one per t-block), then x*s + m2 with s=rstd, m2=-mean*rstd
                broadcast along partitions. The LN gain/bias are folded into
                the consuming matmul's weights/biases host-side.
                """
                sq = [
                    bigp.tile([128, t], BF16, tag="big", name=f"sq{e}")
                    for e in range(ET)
                ]
                xs_ps = ps_s.tile([nt, TB], FP32, tag="stat_x")
                qs_ps = ps_s.tile([nt, TB], FP32, tag="stat_q")
                for tb in range(nt):
                    tbl = slice(TB * tb, TB * (tb + 1))
                    # split x^2 across DVE and ACT (ACT idles in LN phase;
                    # Square shares a table set with exp/ln fillers)
                    nc.vector.tensor_tensor(
                        out=sq[0][:, tbl], in0=src[0][:, tbl],
                        in1=src[0][:, tbl], op=ALU.mult,
                    )
                    nc.scalar.activation(
                        out=sq[1][:, tbl], in_=src[1][:, tbl], func=AF.Square,
                    )
                    for e in range(ET):
                        nc.tensor.matmul(
                            xs_ps[:, :],
                            eyeblk[:, tb, :],
                            src[e][:, tbl],
                            start=(tb == 0 and e == 0),
                            stop=(tb == nt - 1 and e == ET - 1),
                        )
                    for e in range(ET):
                        nc.tensor.matmul(
                            qs_ps[:, :],
                            eyeblk[:, tb, :],
                            sq[e][:, tbl],
                            start=(tb == 0 and e == 0),
                            stop=(tb == nt - 1 and e == ET - 1),
                        )
                mean4 = smallp.tile([nt, TB], FP32, tag="mean4", name="mean4")
                msq4 = smallp.tile([nt, TB], FP32, tag="msq4", name="msq4")
                var4 = smallp.tile([nt, TB], FP32, tag="var4", name="var4")
                s4 = smallp.tile([nt, TB], FP32R, tag="s4", name="s4")
                xs_rows = xs_ps[:, :]
                qs_rows = qs_ps[:, :]
                nc.vector.tensor_scalar(mean4[:, :], xs_rows, 1.0 / E, None, ALU.mult)
                nc.vector.tensor_scalar(msq4[:, :], qs_rows, 1.0 / E, None, ALU.mult)
                nc.vector.tensor_tensor(
                    out=var4[:, :], in0=mean4[:, :], in1=mean4[:, :], op=ALU.mult
                )
                nc.vector.tensor_tensor(
                    out=var4[:, :], in0=msq4[:, :], in1=var4[:, :], op=ALU.subtract
                )
                nc.scalar.activation(out=var4[:, :], in_=var4[:, :], func=AF.Ln, bias=epst[0:nt, :])
                nc.scalar.activation(out=s4[:, :], in_=var4[:, :], func=AF.Exp, scale=-0.5)
                m24 = smallp.tile([nt, TB], FP32R, tag="msq4", name="m24")
                nc.vector.scalar_tensor_tensor(
                    out=m24[:, :], in0=mean4[:, :], scalar=-1.0, in1=s4[:, :],
                    op0=ALU.mult, op1=ALU.mult,
                )
                for tb in range(nt):
                    s_bc = ps_s.tile([128, TB], FP32, tag="stat_x", name="s_bc")
                    m_bc = ps_s.tile([128, TB], FP32, tag="stat_q", name="m_bc")
                    nc.tensor.matmul(
                        s_bc[:, :], selt[:, tb, :], s4[:, :],
                        start=True, stop=True,
                    )
                    nc.tensor.matmul(
                        m_bc[:, :], selt[:, tb, :], m24[:, :],
                        start=True, stop=True,
                    )
                    # stage broadcasts to bf16 SBUF once per t-block so the
                    # per-e apply runs in the DVE 2x bf16 mode
                    s_sb = tmpp.tile([128, TB], BF16, tag="lntmp", name="s_sb")
                    m_sb = tmpp.tile([128, TB], BF16, tag="rbc", name="m_sb")
                    nc.scalar.copy(out=s_sb, in_=s_bc[:, :])
                    nc.scalar.copy(out=m_sb, in_=m_bc[:, :])
                    for e in range(ET):
                        tmp = bigp.tile([128, TB], BF16, tag="lnt2", name="lntmp2")
                        nc.vector.tensor_tensor(
                            out=tmp,
                            in0=src[e][:, TB * tb : TB * (tb + 1)],
                            in1=s_sb, op=ALU.mult,
                        )
                        nc.vector.tensor_tensor(
                            out=out_tiles[e][:, TB * tb : TB * (tb + 1)],
                            in0=tmp, in1=m_sb, op=ALU.add,
                        )

            # ================= layers =================
            for l in range(layers):
                wq_t = [wpool2.tile([128, 6 * HD], BF16, tag=f"wqkv{e}", name=f"wqkv{e}") for e in range(ET)]
                wo_t = [wpool2.tile([128, E], BF16, tag=f"wo{e}", name=f"wot{e}") for e in range(ET)]
                w1_t = [wpool1.tile([128, FF], BF16, tag=f"w1{e}", name=f"w1t{e}") for e in range(ET)]
                w2_t = wpool1.tile([128, UT, E], BF16, tag="w2")
                vec = wpool2.tile([128, 20], FP32, tag="vec")
                for e in range(ET):
                    nc.sync.dma_start(out=wq_t[e], in_=wqkv[l, :, e, :])
                    nc.sync.dma_start(out=wo_t[e], in_=wo_p[l, :, e, :])
                    nc.sync.dma_start(out=w1_t[e], in_=w1_p[l, :, e, :])
                nc.sync.dma_start(out=w2_t, in_=w2_p[l, :, :, :])
                nc.sync.dma_start(out=vec, in_=vecs[l, :, :])

                xln = [xlnp.tile([128, t], BF16, tag=f"xln{e}", name=f"xln{e}") for e in range(ET)]
                layernorm(xT, out_tiles=xln)

                qT = actp.tile([2 * HD, t], BF16, tag="qT")
                kT = actp.tile([2 * HD, t], BF16, tag="kT")
                for tb in range(nt):
                    tsl = slice(TB * tb, TB * (tb + 1))
                    qp = ps_m.tile([2 * HD, TB], FP32, tag="m")
                    for e in range(ET):
                        nc.tensor.matmul(
                            qp[:, :], wq_t[e][:, 0 : 2 * HD], xln[e][:, tsl],
                            start=(e == 0), stop=(e == ET - 1),
                        )
                    # +cq: the ln1_b contribution to q, folded host-side
                    nc.vector.tensor_scalar(
                        qT[:, tsl], qp[:, :], vec[0 : 2 * HD, 0:1], None, ALU.add
                    )
                    kp = ps_m.tile([2 * HD, TB], FP32, tag="m", name="kp")
                    for e in range(ET):
                        nc.tensor.matmul(
                            kp[:, :], wq_t[e][:, 2 * HD : 4 * HD], xln[e][:, tsl],
                            start=(e == 0), stop=(e == ET - 1),
                        )
                    nc.scalar.activation(
                        out=kT[:, tsl], in_=kp[:, :], func=AF.Identity,
                        bias=vec[0 : 2 * HD, 1:2],
                    )
                    for i in range(4 * tb, 4 * tb + 4):
                        vp = ps_o.tile([128, 2, HD], FP32, tag="o", name="vp")
                        for e in range(ET):
                            nc.tensor.matmul(
                                vp[:, :, :],
                                xln[e][:, SC * i : SC * (i + 1)],
                                wq_t[e][:, 4 * HD : 6 * HD],
                                start=(e == 0), stop=(e == ET - 1),
                            )
                        nc.any.tensor_copy(out=vt[:, i, :, 0:HD], in_=vp[:, :, :])

                    # ---- attention for this t-block (qkv ready up to here) ----
                    if "attn" in ablate:
                        if tb == 0:
                            for h in range(HPC):
                                nc.vector.memset(oTp[h].bitcast(FP16), 1.0)
                        continue
                    # both heads' o (+row-sum) packed in one PSUM bank:
                    # head h occupies partitions [64h, 64h+33)
                    op_ps = ps_o.tile([128, TB], FP32, tag="o", name="op_ps")
                    nmax = 4 * tb + 4

                    def emit_o(i, h, exh, d):
                        nc.tensor.matmul(
                            op_ps[64 * h : 64 * h + HD + 1, d:TB],
                            vt[:, i, h, :],
                            exh[:, d:TB],
                            start=(i == 0), stop=(i == nmax - 1),
                            tile_position=(0, 64 * h),
                        )

                    pend = ()
                    for i in range(nmax):
                        d = max(0, SC * i - TB * tb)
                        psl = slice(d, TB)
                        tgl = slice(TB * tb + d, TB * (tb + 1))
                        cur = []
                        for h in range(HPC):
                            rsl = slice(32 * h, 32 * (h + 1))
                            at_ps = ps_a.tile([128, TB], FP32, tag="att", name=f"at_ps{h}")
                            exh = expp.tile([128, TB], BF16, tag=f"exp{h}", name="exh")
                            nc.tensor.matmul(
                                at_ps[:, psl],
                                kT[rsl, SC * i : SC * (i + 1)],
                                qT[rsl, tgl],
                                start=True, stop=True,
                                tile_position=(32 * h, 0),
                            )
                            nc.scalar.activation(
                                out=exh[:, psl], in_=at_ps[:, psl],
                                func=AF.Exp, scale=float(E) ** -0.5,
                            )
                            if i >= 4 * tb:  # diagonal chunk: mask upper triangle
                                nc.vector.tensor_tensor(
                                    out=exh[:, d : d + SC],
                                    in0=exh[:, d : d + SC],
                                    in1=mask[:, :], op=ALU.mult,
                                )
                            cur.append((i, h, exh, d))
                        for ent in pend:
                            emit_o(*ent)
                        pend = cur
                    for ent in pend:
                        emit_o(*ent)
                    # normalize each head by its row-sum (psum row 64h+32):
                    # reciprocal -> K=1 ones-matmul broadcast across HD
                    # partitions (PSUM) -> multiply
                    sr = smallp.tile([HD + 1, TB], FP32, tag="srow", name="sr")
                    for h in range(HPC):
                        # DVE outputs must start at a 32-aligned partition
                        nc.vector.reciprocal(
                            out=sr[HD * h : HD * h + 1, :],
                            in_=op_ps[64 * h + HD : 64 * h + HD + 1, :],
                        )
                        rec_ps = ps_s.tile(
                            [HD, TB], FP32,
                            tag=("stat_x" if h == 0 else "stat_q"), name="rec_ps",
                        )
                        nc.tensor.matmul(
                            rec_ps[:, :],
                            ones33[HD * h : HD * h + 1, :],
                            sr[HD * h : HD * h + 1, :],
                            start=True, stop=True,
                        )
                        rec_sb = tmpp.tile([HD, TB], BF16, tag="rbc", name="rec_sb")
                        nc.vector.tensor_copy(out=rec_sb, in_=rec_ps[:, :])
                        nc.vector.tensor_tensor(
                            out=oTp[h][:, TB * tb : TB * (tb + 1)],
                            in0=op_ps[64 * h : 64 * h + HD, :],
                            in1=rec_sb,
                            op=ALU.mult,
                        )
                        # stream this t-block's slice to the AllGather bounce
                        # buffer now, so the collective input is ready the
                        # moment the last block finishes
                        nc.sync.dma_start(
                            out=cc_in[l][HD * h : HD * (h + 1), TB * tb : TB * (tb + 1)],
                            in_=oTp[h][:, TB * tb : TB * (tb + 1)],
                        )

                # ---- AllGather heads across the 4-core batch group ----
                oT = [actp.tile([128, t], BF16, tag=tg, name=f"oT_{tg}") for tg in ("qT", "kT")]
                if use_collective:
                    nc.gpsimd.collective_compute(
                        "AllGather", ALU.bypass,
                        replica_groups=groups,
                        ins=[cc_in[l][:, :]], outs=[cc_out[l][:, :]],
                    )
                    for e in range(ET):
                        nc.sync.dma_start(
                            out=oT[e], in_=cc_out[l][128 * e : 128 * (e + 1), :]
                        )
                else:
                    # no-comm build (used for TimelineSim): same bounce DMAs as
                    # the real path so DMA time is modeled; cc_out carries
                    # no meaningful data (timing-only build)
                    for e in range(ET):
                        nc.sync.dma_start(
                            out=oT[e], in_=cc_out[l][128 * e : 128 * (e + 1), :]
                        )

                # ---- wo projection + residual ----
                for tb in range(nt):
                    tsl = slice(TB * tb, TB * (tb + 1))
                    for eo in range(ET):
                        wpool = ps_m if eo % 2 == 0 else ps_a
                        wp = wpool.tile([128, TB], FP32, tag="m" if eo % 2 == 0 else "att")
                        for e in range(ET):
                            nc.tensor.matmul(
                                wp[:, :],
                                wo_t[e][:, 128 * eo : 128 * (eo + 1)],
                                oT[e][:, tsl],
                                start=(e == 0), stop=(e == ET - 1),
                            )
                        nc.vector.scalar_tensor_tensor(
                            out=xT[eo][:, tsl], in0=wp[:, :],
                            scalar=vec[:, 8 + eo : 9 + eo], in1=xT[eo][:, tsl],
                            op0=ALU.add, op1=ALU.add,
                        )

                # ---- FFN ----
                xln2 = [xlnp.tile([128, t], BF16, tag=f"xln{e}", name=f"xln{e}") for e in range(ET)]
                layernorm(xT, out_tiles=xln2)
                for tb in range(nt):
                    if "ffn" in ablate:
                        break
                    tsl = slice(TB * tb, TB * (tb + 1))
                    ru_halves = []
                    for half in range(2):
                        ru = bigp.tile([128, UT // 2, TB], BF16, tag="big", name="ru")
                        for uu in range(UT // 2):
                            ut = half * (UT // 2) + uu
                            upool = ps_a if uu % 2 == 0 else ps_o
                            up = upool.tile([128, TB], FP32, tag="att" if uu % 2 == 0 else "o", name="up")
                            for e in range(ET):
                                nc.tensor.matmul(
                                    up[:, :],
                                    w1_t[e][:, 128 * ut : 128 * (ut + 1)],
                                    xln2[e][:, tsl],
                                    start=(e == 0), stop=(e == ET - 1),
                                )
                            nc.scalar.activation(
                                out=ru[:, uu, :], in_=up[:, :], func=AF.Relu,
                                bias=vec[:, 10 + ut : 11 + ut],
                            )
                        ru_halves.append(ru)
                    for eo in range(ET):
                        wp2 = ps_m.tile([128, TB], FP32, tag="m", name="wp2")
                        for ut in range(UT):
                            nc.tensor.matmul(
                                wp2[:, :],
                                w2_t[:, ut, 128 * eo : 128 * (eo + 1)],
                                ru_halves[ut // (UT // 2)][:, ut % (UT // 2), :],
                                start=(ut == 0), stop=(ut == UT - 1),
                            )
                        nc.vector.scalar_tensor_tensor(
                            out=xT[eo][:, tsl], in0=wp2[:, :],
                            scalar=vec[:, 18 + eo : 19 + eo], in1=xT[eo][:, tsl],
                            op0=ALU.add, op1=ALU.add,
                        )

            # ================= final LN + lm_head =================
            xf = [xlnp.tile([128, t], BF16, tag=f"xln{e}", name=f"xln{e}") for e in range(ET)]
            layernorm(xT, out_tiles=xf)
            GB = min(8, ntc)  # token-chunks batched per logits DMA
            for vb in range(nvb if "lm" not in ablate else 1):
                wh = whp.tile([128, ET, 512], BF16, tag="wh")
                nc.sync.dma_start(out=wh, in_=whead[:, :, 512 * vb : 512 * (vb + 1)])
                for g in range(ntc // GB):
                    lg = lgp.tile([128, GB, 512], FP16, tag="lg")
                    for k in range(GB):
                        tcn = GB * g + k
                        # rotate across three PSUM pools (6 banks) so the
                        # matmul/copy pipeline never waits on a bank
                        lpool = (ps_m, ps_a, ps_o)[k % 3]
                        lp = lpool.tile([128, 512], FP32, tag=("m", "att", "o")[k % 3])
                        for e in range(ET):
                            nc.tensor.matmul(
                                lp[:, :],
                                xf[e][:, 128 * tcn : 128 * (tcn + 1)],
                                wh[:, e, :],
                                start=(e == 0), stop=(e == ET - 1),
                            )
                        if (vb + tcn) % 2 == 0:
                            nc.vector.tensor_copy(out=lg[:, k, :], in_=lp[:, :])
                        else:
                            nc.scalar.copy(out=lg[:, k, :], in_=lp[:, :])
                    nc.sync.dma_start(
                        out=logits[:, GB * g : GB * (g + 1), 512 * vb : 512 * (vb + 1)],
                        in_=lg,
                    )

    nc.compile()
    return nc


# ---------------- host-side prep / unshard ----------------

def prep_core_inputs(c, X, tok_emb, pos_emb, wq, wk, wv, wo, bo, w1, b1, w2, b2,
                     ln1_g, ln1_b, ln2_g, ln2_b, lnf_g, lnf_b, w_head, b_head,
                     t=T, layers=L, vsp=VSP):
    b = c // GROUP
    j = c % GROUP
    heads = [HPC * j + k for k in range(HPC)]

    f32 = np.float32
    Xb = np.asarray(X[b]).astype(np.int64)
    x0 = (np.asarray(tok_emb)[Xb] + np.asarray(pos_emb)[:t]).astype(f32).T  # [E, t]

    wq = np.asarray(wq); wk = np.asarray(wk); wv = np.asarray(wv)
    wqkv_h = np.empty((layers, 128, ET, 6 * HD), f32)
    wo_h = np.empty((layers, 128, ET, E), f32)
    w1_h = np.empty((layers, 128, ET, FF), f32)
    w2_h = np.empty((layers, 128, UT, E), f32)
    vecs_h = np.zeros((layers, 128, 20), f32)
    for l in range(layers):
        # fold LN gains into the consuming weights and LN biases into
        # per-output-constant corrections (exact for affine LN):
        #   xln_true = xln_raw * g + b  =>  W^T xln_true = (gW)^T xln_raw + W^T b
        g1 = np.asarray(ln1_g[l]).astype(f32)[:, None]
        b1n = np.asarray(ln1_b[l]).astype(f32)
        g2 = np.asarray(ln2_g[l]).astype(f32)[:, None]
        b2n = np.asarray(ln2_b[l]).astype(f32)
        qc = np.concatenate([wq[l, h] for h in heads], axis=1)  # [E, 64]
        kc = np.concatenate([wk[l, h] for h in heads], axis=1)
        vc = np.concatenate([wv[l, h] for h in heads], axis=1)
        cq = qc.T @ b1n  # [64] q bias from ln1_b
        ck = kc.T @ b1n
        # v bias from ln1_b for ALL heads, folded through wo into bo
        cv_full = np.concatenate([wv[l, h].T @ b1n for h in range(H)])  # [E]
        bo_eff = np.asarray(bo[l]).astype(f32) + np.asarray(wo[l]).T @ cv_full
        b1_eff = np.asarray(b1[l]).astype(f32) + np.asarray(w1[l]).T @ b2n
        qkv = np.concatenate([qc, kc, vc], axis=1) * g1  # [E, 192]
        wqkv_h[l] = qkv.reshape(ET, 128, 6 * HD).transpose(1, 0, 2)
        wo_h[l] = np.asarray(wo[l]).reshape(ET, 128, E).transpose(1, 0, 2)
        w1_h[l] = (np.asarray(w1[l]) * g2).reshape(ET, 128, FF).transpose(1, 0, 2)
        w2_h[l] = np.asarray(w2[l]).reshape(UT, 128, E).transpose(1, 0, 2)
        vecs_h[l, 0:2 * HD, 0] = cq
        vecs_h[l, 0:2 * HD, 1] = ck
        vecs_h[l, :, 8:10] = bo_eff.reshape(2, 128).T
        vecs_h[l, :, 10:18] = b1_eff.reshape(8, 128).T
        vecs_h[l, :, 18:20] = np.asarray(b2[l]).astype(f32).reshape(2, 128).T

    w_head = np.asarray(w_head) * np.asarray(lnf_g).astype(f32)[:, None]
    vs = w_head.shape[1] // GROUP
    wh = np.zeros((E, vsp), f32)
    wh[:, :vs] = w_head[:, vs * j : vs * (j + 1)]
    whead_h = np.ascontiguousarray(wh.reshape(ET, 128, vsp).transpose(1, 0, 2))

    sp = np.arange(SC)[:, None]
    tp = np.arange(SC)[None, :]
    mask_h = (sp <= tp).astype(f32)

    nt = t // TB
    nsc = t // SC
    peye_h = np.zeros((128, nt, nt), f32)
    for tb in range(nt):
        peye_h[:, tb, tb] = 1.0
    vtc_h = np.ones((128, nsc, 2), f32)

    bf = NP_BF16
    return {
        "x0": np.ascontiguousarray(x0).astype(bf),
        "wqkv": np.ascontiguousarray(wqkv_h).astype(bf),
        "wo": np.ascontiguousarray(wo_h).astype(bf),
        "w1": np.ascontiguousarray(w1_h).astype(bf),
        "w2": np.ascontiguousarray(w2_h).astype(bf),
        "vecs": np.ascontiguousarray(vecs_h),
        "whead": whead_h.astype(bf),
        "mask": mask_h.astype(bf),
        "peye": peye_h.astype(bf),
        "vtc": vtc_h.astype(bf),
        "selp": np.ascontiguousarray(
            np.broadcast_to(np.eye(nt, dtype=f32)[:, :, None], (nt, nt, 128))
        ),
    }


_NC_CACHE = {}


def _get_nc():
    if "nc" not in _NC_CACHE:
        _NC_CACHE["nc"] = build_nc()
    return _NC_CACHE["nc"]


def kernel(**inputs):
    nc = _get_nc()
    in_maps = [prep_core_inputs(c, **inputs) for c in range(NCORES)]
    res = run_bass_kernel_spmd(nc, in_maps, list(range(NCORES)))
    out = np.empty((B, T, V), np.float32)
    for c in range(NCORES):
        b, j = c // GROUP, c % GROUP
        lg = res.results[c]["logits"]  # [128, T//128, VSP]
        lg = lg.transpose(1, 0, 2).reshape(T, VSP)
        out[b, :, VS * j : VS * (j + 1)] = lg[:, :VS].astype(np.float32)
    # b_head plus the final-LN bias folded through w_head (host-side)
    bh_eff = np.asarray(inputs["b_head"]).astype(np.float32) + (
        np.asarray(inputs["w_head"]).astype(np.float32).T
        @ np.asarray(inputs["lnf_b"]).astype(np.float32)
    )
    if np.any(bh_eff):
        out += bh_eff[None, None, :]
    return out


# revision 50
# speedup vs baseline: 1.3293x; 1.0094x over previous
"""Trainium2 Bass kernel for a 4-layer bigram-LM dense transformer.

Full-model shapes: B=2, T=2048, E=256, H=8, L=4, V=32000.

Sharding over 8 NeuronCores (self-contained, hardcoded):
  - 2-way data parallel over batch: cores 0-3 handle batch 0, cores 4-7
    batch 1 (a "batch group" of 4 cores each).
  - Within a batch group, per-token work (LN / QKV / wo / FFN) is
    replicated; attention (the exp-heavy part) is sharded 2 heads/core
    and re-assembled with one 4-rank AllGather per layer (bf16 payload).
  - lm_head is sharded 4-way over vocab columns within the group
    (8000 cols/core, padded to 8192), so the dominant logits write is
    split 8 ways and emitted as fp16 (host converts back to fp32).

Compute layout: activations live transposed [E, T] in SBUF so every
matmul contracts over the partition axis with zero transposes. All
activations and weights are bf16 (PSUM accumulation stays fp32), which
doubles DVE elementwise throughput and halves HBM/collective traffic.
LN gains are folded into the consuming weights host-side and LN biases
become per-output constants (q/k copy biases, bo/b1/b_head), so the LN
apply is just two bf16 tensor_tensor ops. Softmax skips the
max-subtraction (scores are ~1e-1 scale; exp cannot overflow) and row
sums ride along in the attention-output matmul via a ones column packed
next to V; the 1/rowsum is fanned across partitions with a K=1
ones-matmul (no DRAM round-trip). The two heads' score matmuls are
interleaved so they occupy different 32-row PE strips (tile_position)
and run concurrently; the two attn@V matmuls are packed into one PSUM
bank at column offsets 0 and 64 and also run concurrently. Logits are
emitted fp16 in a [128, T/128, V-shard] layout so eight token-chunks
batch into each DMA (the HWDGE fixed cost per descriptor-set is ~625ns),
and the lm_head matmul/copy pipeline rotates across four PSUM banks.
"""

import numpy as np
import ml_dtypes

import concourse.bass as bass
import concourse.mybir as mybir
import concourse.tile as tile
from concourse import bacc
from concourse.bass_utils import run_bass_kernel_spmd

AF = mybir.ActivationFunctionType
ALU = mybir.AluOpType
FP32 = mybir.dt.float32
FP32R = mybir.dt.float32r
BF16 = mybir.dt.bfloat16
FP16 = mybir.dt.float16
NP_BF16 = ml_dtypes.bfloat16

# model dims (full problem)
B, T, E, H, L, V = 2, 2048, 256, 8, 4, 32000
HD = E // H  # 32
EPS = 1e-5
NCORES = 8
GROUP = 4  # cores per batch group
HPC = H // GROUP  # heads per core (2)
VS = V // GROUP  # vocab shard per core (8000)
VSP = 8192  # padded vocab shard
TB = 512  # t-block (PSUM bank free dim)
SC = 128  # s-chunk (partition dim)
ET = E // 128  # embedding partition tiles (2)
FF = 4 * E  # 1024
UT = FF // 128  # ffn u-tiles (8)


def build_nc(t=T, layers=L, vsp=VSP, use_collective=True, ablate=()):
    """Build + compile the per-core Bass program (SPMD: same program, 8 cores)."""
    nt = t // TB      # t-blocks
    nsc = t // SC     # s-chunks
    ntc = t // 128    # t-chunks for lm head
    nvb = vsp // 512  # vocab blocks

    nc = bacc.Bacc("TRN2", num_devices=NCORES)

    # ---- DRAM parameters (per core) ----
    x0 = nc.declare_dram_parameter("x0", [E, t], BF16, isOutput=False)
    wqkv = nc.declare_dram_parameter("wqkv", [layers, 128, ET, 6 * HD], BF16, isOutput=False)
    wo_p = nc.declare_dram_parameter("wo", [layers, 128, ET, E], BF16, isOutput=False)
    w1_p = nc.declare_dram_parameter("w1", [layers, 128, ET, FF], BF16, isOutput=False)
    w2_p = nc.declare_dram_parameter("w2", [layers, 128, UT, E], BF16, isOutput=False)
    vecs = nc.declare_dram_parameter("vecs", [layers, 128, 20], FP32, isOutput=False)
    whead = nc.declare_dram_parameter("whead", [128, ET, vsp], BF16, isOutput=False)
    maskp = nc.declare_dram_parameter("mask", [SC, SC], BF16, isOutput=False)
    peye = nc.declare_dram_parameter("peye", [128, nt, nt], BF16, isOutput=False)
    vtc = nc.declare_dram_parameter("vtc", [128, nsc, 2], BF16, False)
    selp = nc.declare_dram_parameter("selp", [nt, nt, 128], FP32R, isOutput=False)
    logits = nc.declare_dram_parameter("logits", [128, t // 128, vsp], FP16, isOutput=True)

    # internal DRAM bounce buffers for the per-layer AllGather (bf16)
    cc_in = [nc.dram_tensor(f"cc_in{l}", [HPC * HD, t], BF16) for l in range(layers)]
    cc_out = [nc.dram_tensor(f"cc_out{l}", [GROUP * HPC * HD, t], BF16) for l in range(layers)]
    groups = [[0, 1, 2, 3], [4, 5, 6, 7]]

    from contextlib import ExitStack
    with tile.TileContext(nc) as tc:
        with ExitStack() as _ctx:
            persist = _ctx.enter_context(tc.tile_pool(name="persist", bufs=1))
            wpool2 = _ctx.enter_context(tc.tile_pool(name="wpool2", bufs=2))
            wpool1 = _ctx.enter_context(tc.tile_pool(name="wpool1", bufs=2))
            actp = _ctx.enter_context(tc.tile_pool(name="actp", bufs=1))
            xlnp = _ctx.enter_context(tc.tile_pool(name="xlnp", bufs=2))
            bigp = _ctx.enter_context(tc.tile_pool(name="bigp", bufs=3))
            expp = _ctx.enter_context(tc.tile_pool(name="expp", bufs=4))
            smallp = _ctx.enter_context(tc.tile_pool(name="smallp", bufs=3))
            tmpp = _ctx.enter_context(tc.tile_pool(name="tmpp", bufs=3))
            lgp = _ctx.enter_context(tc.tile_pool(name="lgp", bufs=3))
            whp = _ctx.enter_context(tc.tile_pool(name="whp", bufs=16))
            dpool = _ctx.enter_context(tc.tile_pool(name="dpool", bufs=2, space="DRAM"))
            ps_a = _ctx.enter_context(tc.tile_pool(name="ps_a", bufs=2, space="PSUM"))
            ps_o = _ctx.enter_context(tc.tile_pool(name="ps_o", bufs=2, space="PSUM"))
            ps_m = _ctx.enter_context(tc.tile_pool(name="ps_m", bufs=2, space="PSUM"))
            ps_s = _ctx.enter_context(tc.tile_pool(name="ps_s", bufs=1, space="PSUM"))
            # ---- persistent tiles ----
            xT = [persist.tile([128, t], BF16, tag=f"xT{e}", name=f"xT{e}") for e in range(ET)]
            for e in range(ET):
                nc.sync.dma_start(out=xT[e], in_=x0[128 * e : 128 * (e + 1), :])
            mask = persist.tile([SC, SC], BF16, tag="mask")
            nc.sync.dma_start(out=mask, in_=maskp[:, :])
            # v tile: per chunk cols = [vA(32) | ones | vB(32) | ones] so the
            # 33-wide per-head lhsT computes o rows 0:32 plus a row-sum row 32
            vt = persist.tile([128, nsc, 2, HD + 1], BF16, tag="vt")
            nc.sync.dma_start(out=vt[:, :, :, HD : HD + 1], in_=vtc[:, :, :])
            eyeblk = persist.tile([128, nt, nt], BF16, tag="eyeblk")
            nc.sync.dma_start(out=eyeblk, in_=peye[:, :, :])
            selt = persist.tile([nt, nt, 128], FP32R, tag="selt")
            nc.sync.dma_start(out=selt, in_=selp[:, :, :])
            # own heads' normalized attention out, pre-AllGather, [32, t] each
            oTp = [persist.tile([HD, t], BF16, tag=f"oTp{h}", name=f"oTp{h}") for h in range(HPC)]
            epst = persist.tile([128, 1], FP32, tag="epst")
            nc.vector.memset(epst, EPS)
            # ones rows at partitions 0 and 32: lhsT for the K=1 broadcast
            # matmul that fans the per-token 1/rowsum out to HD partitions
            ones33 = persist.tile([HD + 1, HD], FP32, tag="ones33")
            nc.vector.memset(ones33, 1.0)

            def layernorm(src, out_tiles):
                if "ln" in ablate:
                    for e in range(ET):
                        nc.scalar.activation(
                            out=out_tiles[e][:, :], in_=src[e][:, :], func=AF.Identity,
                        )
                    return
                """src: list of ET [128, t] bf16 tiles -> out_tiles bf16.

                Per-token stats via ones-matmuls into PSUM rows {0,32,64,96}
                (one per t-block), then x*s + m2 with s=rstd, m2=-mean*rstd
                broadcast along partitions. The LN gain/bias are folded into
                the consuming matmul's weights/biases host-side.
                """
                sq = [
                    bigp.tile([128, t], BF16, tag="big", name=f"sq{e}")
                    for e in range(ET)
                ]
                xs_ps = ps_s.tile([nt, TB], FP32, tag="stat_x")
                qs_ps = ps_s.tile([nt, TB], FP32, tag="stat_q")
                for tb in range(nt):
                    tbl = slice(TB * tb, TB * (tb + 1))
                    # split x^2 across DVE and ACT (ACT idles in LN phase;
                    # Square shares a table set with exp/ln fillers)
                    nc.vector.tensor_tensor(
                        out=sq[0][:, tbl], in0=src[0][:, tbl],
                        in1=src[0][:, tbl], op=ALU.mult,
                    )
                    nc.scalar.activation(
                        out=sq[1][:, tbl], in_=src[1][:, tbl], func=AF.Square,
                    )
                    for e in range(ET):
                        nc.tensor.matmul(
                            xs_ps[:, :],
                            eyeblk[:, tb, :],
                            src[e][:, tbl],
                            start=(tb == 0 and e == 0),
                            stop=(tb == nt - 1 and e == ET - 1),
                        )
                    for e in range(ET):
                        nc.tensor.matmul(
                            qs_ps[:, :],
                            eyeblk[:, tb, :],
                            sq[e][:, tbl],
                            start=(tb == 0 and e == 0),
                            stop=(tb == nt - 1 and e == ET - 1),
                        )
                mean4 = smallp.tile([nt, TB], FP32, tag="mean4", name="mean4")
                msq4 = smallp.tile([nt, TB], FP32, tag="msq4", name="msq4")
                var4 = smallp.tile([nt, TB], FP32, tag="var4", name="var4")
                s4 = smallp.tile([nt, TB], FP32R, tag="s4", name="s4")
                xs_rows = xs_ps[:, :]
                qs_rows = qs_ps[:, :]
                nc.vector.tensor_scalar(mean4[:, :], xs_rows, 1.0 / E, None, ALU.mult)
                nc.vector.tensor_scalar(msq4[:, :], qs_rows, 1.0 / E, None, ALU.mult)
                nc.vector.tensor_tensor(
                    out=var4[:, :], in0=mean4[:, :], in1=mean4[:, :], op=ALU.mult
                )
                nc.vector.tensor_tensor(
                    out=var4[:, :], in0=msq4[:, :], in1=var4[:, :], op=ALU.subtract
                )
                nc.scalar.activation(out=var4[:, :], in_=var4[:, :], func=AF.Ln, bias=epst[0:nt, :])
                nc.scalar.activation(out=s4[:, :], in_=var4[:, :], func=AF.Exp, scale=-0.5)
                m24 = smallp.tile([nt, TB], FP32R, tag="msq4", name="m24")
                nc.vector.scalar_tensor_tensor(
                    out=m24[:, :], in0=mean4[:, :], scalar=-1.0, in1=s4[:, :],
                    op0=ALU.mult, op1=ALU.mult,
                )
                for tb in range(nt):
                    s_bc = ps_s.tile([128, TB], FP32, tag="stat_x", name="s_bc")
                    m_bc = ps_s.tile([128, TB], FP32, tag="stat_q", name="m_bc")
                    nc.tensor.matmul(
                        s_bc[:, :], selt[:, tb, :], s4[:, :],
                        start=True, stop=True,
                    )
                    nc.tensor.matmul(
                        m_bc[:, :], selt[:, tb, :], m24[:, :],
                        start=True, stop=True,
                    )
                    # stage broadcasts to bf16 SBUF once per t-block so the
                    # per-e apply runs in the DVE 2x bf16 mode
                    s_sb = tmpp.tile([128, TB], BF16, tag="lntmp", name="s_sb")
                    m_sb = tmpp.tile([128, TB], BF16, tag="rbc", name="m_sb")
                    nc.scalar.copy(out=s_sb, in_=s_bc[:, :])
                    nc.scalar.copy(out=m_sb, in_=m_bc[:, :])
                    for e in range(ET):
                        tmp = bigp.tile([128, TB], BF16, tag="lnt2", name="lntmp2")
                        nc.vector.tensor_tensor(
                            out=tmp,
                            in0=src[e][:, TB * tb : TB * (tb + 1)],
                            in1=s_sb, op=ALU.mult,
                        )
                        nc.vector.tensor_tensor(
                            out=out_tiles[e][:, TB * tb : TB * (tb + 1)],
                            in0=tmp, in1=m_sb, op=ALU.add,
                        )

            # ================= layers =================
            for l in range(layers):
                wq_t = [wpool2.tile([128, 6 * HD], BF16, tag=f"wqkv{e}", name=f"wqkv{e}") for e in range(ET)]
                wo_t = [wpool2.tile([128, E], BF16, tag=f"wo{e}", name=f"wot{e}") for e in range(ET)]
                w1_t = [wpool1.tile([128, FF], BF16, tag=f"w1{e}", name=f"w1t{e}") for e in range(ET)]
                w2_t = wpool1.tile([128, UT, E], BF16, tag="w2")
                vec = wpool2.tile([128, 20], FP32, tag="vec")
                for e in range(ET):
                    nc.sync.dma_start(out=wq_t[e], in_=wqkv[l, :, e, :])
                    nc.sync.dma_start(out=wo_t[e], in_=wo_p[l, :, e, :])
                    nc.sync.dma_start(out=w1_t[e], in_=w1_p[l, :, e, :])
                nc.sync.dma_start(out=w2_t, in_=w2_p[l, :, :, :])
                nc.sync.dma_start(out=vec, in_=vecs[l, :, :])

                xln = [xlnp.tile([128, t], BF16, tag=f"xln{e}", name=f"xln{e}") for e in range(ET)]
                layernorm(xT, out_tiles=xln)

                qT = actp.tile([2 * HD, t], BF16, tag="qT")
                kT = actp.tile([2 * HD, t], BF16, tag="kT")
                for tb in range(nt):
                    tsl = slice(TB * tb, TB * (tb + 1))
                    qp = ps_m.tile([2 * HD, TB], FP32, tag="m")
                    for e in range(ET):
                        nc.tensor.matmul(
                            qp[:, :], wq_t[e][:, 0 : 2 * HD], xln[e][:, tsl],
                            start=(e == 0), stop=(e == ET - 1),
                        )
                    # +cq: the ln1_b contribution to q, folded host-side
                    nc.vector.tensor_scalar(
                        qT[:, tsl], qp[:, :], vec[0 : 2 * HD, 0:1], None, ALU.add
                    )
                    kp = ps_m.tile([2 * HD, TB], FP32, tag="m", name="kp")
                    for e in range(ET):
                        nc.tensor.matmul(
                            kp[:, :], wq_t[e][:, 2 * HD : 4 * HD], xln[e][:, tsl],
                            start=(e == 0), stop=(e == ET - 1),
                        )
                    nc.scalar.activation(
                        out=kT[:, tsl], in_=kp[:, :], func=AF.Identity,
                        bias=vec[0 : 2 * HD, 1:2],
                    )
                    for i in range(4 * tb, 4 * tb + 4):
                        vp = ps_o.tile([128, 2, HD], FP32, tag="o", name="vp")
                        for e in range(ET):
                            nc.tensor.matmul(
                                vp[:, :, :],
                                xln[e][:, SC * i : SC * (i + 1)],
                                wq_t[e][:, 4 * HD : 6 * HD],
                                start=(e == 0), stop=(e == ET - 1),
                            )
                        nc.any.tensor_copy(out=vt[:, i, :, 0:HD], in_=vp[:, :, :])

                    # ---- attention for this t-block (qkv ready up to here) ----
                    if "attn" in ablate:
                        if tb == 0:
                            for h in range(HPC):
                                nc.vector.memset(oTp[h].bitcast(FP16), 1.0)
                        continue
                    # both heads' o (+row-sum) packed in one PSUM bank:
                    # head h occupies partitions [64h, 64h+33)
                    op_ps = ps_o.tile([128, TB], FP32, tag="o", name="op_ps")
                    nmax = 4 * tb + 4

                    def emit_o(i, h, exh, d):
                        nc.tensor.matmul(
                            op_ps[64 * h : 64 * h + HD + 1, d:TB],
                            vt[:, i, h, :],
                            exh[:, d:TB],
                            start=(i == 0), stop=(i == nmax - 1),
                            tile_position=(0, 64 * h),
                        )

                    pend = ()
                    for i in range(nmax):
                        d = max(0, SC * i - TB * tb)
                        psl = slice(d, TB)
                        tgl = slice(TB * tb + d, TB * (tb + 1))
                        cur = []
                        for h in range(HPC):
                            rsl = slice(32 * h, 32 * (h + 1))
                            at_ps = ps_a.tile([128, TB], FP32, tag="att", name=f"at_ps{h}")
                            exh = expp.tile([128, TB], BF16, tag=f"exp{h}", name="exh")
                            nc.tensor.matmul(
                                at_ps[:, psl],
                                kT[rsl, SC * i : SC * (i + 1)],
                                qT[rsl, tgl],
                                start=True, stop=True,
                                tile_position=(32 * h, 0),
                            )
                            nc.scalar.activation(
                                out=exh[:, psl], in_=at_ps[:, psl],
                                func=AF.Exp, scale=float(E) ** -0.5,
                            )
                            if i >= 4 * tb:  # diagonal chunk: mask upper triangle
                                nc.vector.tensor_tensor(
                                    out=exh[:, d : d + SC],
                                    in0=exh[:, d : d + SC],
                                    in1=mask[:, :], op=ALU.mult,
                                )
                            cur.append((i, h, exh, d))
                        for ent in pend:
                            emit_o(*ent)
                        pend = cur
                    for ent in pend:
                        emit_o(*ent)
                    # normalize each head by its row-sum (psum row 64h+32):
                    # reciprocal -> K=1 ones-matmul broadcast across HD
                    # partitions (PSUM) -> multiply
                    sr = smallp.tile([HD + 1, TB], FP32, tag="srow", name="sr")
                    for h in range(HPC):
                        # DVE outputs must start at a 32-aligned partition
                        nc.vector.reciprocal(
                            out=sr[HD * h : HD * h + 1, :],
                            in_=op_ps[64 * h + HD : 64 * h + HD + 1, :],
                        )
                        rec_ps = ps_s.tile(
                            [HD, TB], FP32,
                            tag=("stat_x" if h == 0 else "stat_q"), name="rec_ps",
                        )
                        nc.tensor.matmul(
                            rec_ps[:, :],
                            ones33[HD * h : HD * h + 1, :],
                            sr[HD * h : HD * h + 1, :],
                            start=True, stop=True,
                        )
                        rec_sb = tmpp.tile([HD, TB], BF16, tag="rbc", name="rec_sb")
                        nc.vector.tensor_copy(out=rec_sb, in_=rec_ps[:, :])
                        nc.vector.tensor_tensor(
                            out=oTp[h][:, TB * tb : TB * (tb + 1)],
                            in0=op_ps[64 * h : 64 * h + HD, :],
                            in1=rec_sb,
                            op=ALU.mult,
                        )
                        # stream this t-block's slice to the AllGather bounce
                        # buffer now, so the collective input is ready the
                        # moment the last block finishes
                        nc.sync.dma_start(
                            out=cc_in[l][HD * h : HD * (h + 1), TB * tb : TB * (tb + 1)],
                            in_=oTp[h][:, TB * tb : TB * (tb + 1)],
                        )

                # ---- AllGather heads across the 4-core batch group ----
                oT = [actp.tile([128, t], BF16, tag=tg, name=f"oT_{tg}") for tg in ("qT", "kT")]
                if use_collective:
                    nc.gpsimd.collective_compute(
                        "AllGather", ALU.bypass,
                        replica_groups=groups,
                        ins=[cc_in[l][:, :]], outs=[cc_out[l][:, :]],
                    )
                    for e in range(ET):
                        nc.sync.dma_start(
                            out=oT[e], in_=cc_out[l][128 * e : 128 * (e + 1), :]
                        )
                else:
                    # no-comm build (used for TimelineSim): same bounce DMAs as
                    # the real path so DMA time is modeled; cc_out carries
                    # no meaningful data (timing-only build)
                    for e in range(ET):
                        nc.sync.dma_start(
                            out=oT[e], in_=cc_out[l][128 * e : 128 * (e + 1), :]
                        )

                # ---- wo projection + residual ----
                for tb in range(nt):
                    tsl = slice(TB * tb, TB * (tb + 1))
                    for eo in range(ET):
                        wpool = ps_m if eo % 2 == 0 else ps_a
                        wp = wpool.tile([128, TB], FP32, tag="m" if eo % 2 == 0 else "att")
                        for e in range(ET):
                            nc.tensor.matmul(
                                wp[:, :],
                                wo_t[e][:, 128 * eo : 128 * (eo + 1)],
                                oT[e][:, tsl],
                                start=(e == 0), stop=(e == ET - 1),
                            )
                        nc.vector.scalar_tensor_tensor(
                            out=xT[eo][:, tsl], in0=wp[:, :],
                            scalar=vec[:, 8 + eo : 9 + eo], in1=xT[eo][:, tsl],
                            op0=ALU.add, op1=ALU.add,
                        )

                # ---- FFN ----
                xln2 = [xlnp.tile([128, t], BF16, tag=f"xln{e}", name=f"xln{e}") for e in range(ET)]
                layernorm(xT, out_tiles=xln2)
                for tb in range(nt):
                    if "ffn" in ablate:
                        break
                    tsl = slice(TB * tb, TB * (tb + 1))
                    ru_halves = []
                    for half in range(2):
                        ru = bigp.tile([128, UT // 2, TB], BF16, tag="big", name="ru")
                        for uu in range(UT // 2):
                            ut = half * (UT // 2) + uu
                            upool = ps_a if uu % 2 == 0 else ps_o
                            up = upool.tile([128, TB], FP32, tag="att" if uu % 2 == 0 else "o", name="up")
                            for e in range(ET):
                                nc.tensor.matmul(
                                    up[:, :],
                                    w1_t[e][:, 128 * ut : 128 * (ut + 1)],
                                    xln2[e][:, tsl],
                                    start=(e == 0), stop=(e == ET - 1),
                                )
                            nc.scalar.activation(
                                out=ru[:, uu, :], in_=up[:, :], func=AF.Relu,
                                bias=vec[:, 10 + ut : 11 + ut],
                            )
                        ru_halves.append(ru)
                    for eo in range(ET):
                        wp2 = ps_m.tile([128, TB], FP32, tag="m", name="wp2")
                        for ut in range(UT):
                            nc.tensor.matmul(
                                wp2[:, :],
                                w2_t[:, ut, 128 * eo : 128 * (eo + 1)],
                                ru_halves[ut // (UT // 2)][:, ut % (UT // 2), :],
                                start=(ut == 0), stop=(ut == UT - 1),
                            )
                        nc.vector.scalar_tensor_tensor(
                            out=xT[eo][:, tsl], in0=wp2[:, :],
                            scalar=vec[:, 18 + eo : 19 + eo], in1=xT[eo][:, tsl],
                            op0=ALU.add, op1=ALU.add,
                        )

            # ================= final LN + lm_head =================
            xf = [xlnp.tile([128, t], BF16, tag=f"xln{e}", name=f"xln{e}") for e in range(ET)]
            layernorm(xT, out_tiles=xf)
            GB = min(8, ntc)  # token-chunks batched per logits DMA
            # token-group-major: all vocab blocks for the first GB token
            # chunks run as soon as the first half of the final LN lands.
            # Weight tiles are loaded once (g==0) and kept resident.
            nvb_r = nvb if "lm" not in ablate else 1
            whs = []
            for g in range(ntc // GB):
                for vb in range(nvb_r):
                    if g == 0:
                        wh = whp.tile([128, ET, 512], BF16, tag="wh", name=f"wh{vb}")
                        nc.sync.dma_start(out=wh, in_=whead[:, :, 512 * vb : 512 * (vb + 1)])
                        whs.append(wh)
                    else:
                        wh = whs[vb]
                    lg = lgp.tile([128, GB, 512], FP16, tag="lg")
                    for k in range(GB):
                        tcn = GB * g + k
                        # rotate across three PSUM pools (6 banks) so the
                        # matmul/copy pipeline never waits on a bank
                        lpool = (ps_m, ps_a, ps_o)[k % 3]
                        lp = lpool.tile([128, 512], FP32, tag=("m", "att", "o")[k % 3])
                        for e in range(ET):
                            nc.tensor.matmul(
                                lp[:, :],
                                xf[e][:, 128 * tcn : 128 * (tcn + 1)],
                                wh[:, e, :],
                                start=(e == 0), stop=(e == ET - 1),
                            )
                        if (vb + tcn) % 2 == 0:
                            nc.vector.tensor_copy(out=lg[:, k, :], in_=lp[:, :])
                        else:
                            nc.scalar.copy(out=lg[:, k, :], in_=lp[:, :])
                    nc.sync.dma_start(
                        out=logits[:, GB * g : GB * (g + 1), 512 * vb : 512 * (vb + 1)],
                        in_=lg,
                    )

    nc.compile()
    return nc


# ---------------- host-side prep / unshard ----------------

def prep_core_inputs(c, X, tok_emb, pos_emb, wq, wk, wv, wo, bo, w1, b1, w2, b2,
                     ln1_g, ln1_b, ln2_g, ln2_b, lnf_g, lnf_b, w_head, b_head,
                     t=T, layers=L, vsp=VSP):
    b = c // GROUP
    j = c % GROUP
    heads = [HPC * j + k for k in range(HPC)]

    f32 = np.float32
    Xb = np.asarray(X[b]).astype(np.int64)
    x0 = (np.asarray(tok_emb)[Xb] + np.asarray(pos_emb)[:t]).astype(f32).T  # [E, t]

    wq = np.asarray(wq); wk = np.asarray(wk); wv = np.asarray(wv)
    wqkv_h = np.empty((layers, 128, ET, 6 * HD), f32)
    wo_h = np.empty((layers, 128, ET, E), f32)
    w1_h = np.empty((layers, 128, ET, FF), f32)
    w2_h = np.empty((layers, 128, UT, E), f32)
    vecs_h = np.zeros((layers, 128, 20), f32)
    for l in range(layers):
        # fold LN gains into the consuming weights and LN biases into
        # per-output-constant corrections (exact for affine LN):
        #   xln_true = xln_raw * g + b  =>  W^T xln_true = (gW)^T xln_raw + W^T b
        g1 = np.asarray(ln1_g[l]).astype(f32)[:, None]
        b1n = np.asarray(ln1_b[l]).astype(f32)
        g2 = np.asarray(ln2_g[l]).astype(f32)[:, None]
        b2n = np.asarray(ln2_b[l]).astype(f32)
        qc = np.concatenate([wq[l, h] for h in heads], axis=1)  # [E, 64]
        kc = np.concatenate([wk[l, h] for h in heads], axis=1)
        vc = np.concatenate([wv[l, h] for h in heads], axis=1)
        cq = qc.T @ b1n  # [64] q bias from ln1_b
        ck = kc.T @ b1n
        # v bias from ln1_b for ALL heads, folded through wo into bo
        cv_full = np.concatenate([wv[l, h].T @ b1n for h in range(H)])  # [E]
        bo_eff = np.asarray(bo[l]).astype(f32) + np.asarray(wo[l]).T @ cv_full
        b1_eff = np.asarray(b1[l]).astype(f32) + np.asarray(w1[l]).T @ b2n
        qkv = np.concatenate([qc, kc, vc], axis=1) * g1  # [E, 192]
        wqkv_h[l] = qkv.reshape(ET, 128, 6 * HD).transpose(1, 0, 2)
        wo_h[l] = np.asarray(wo[l]).reshape(ET, 128, E).transpose(1, 0, 2)
        w1_h[l] = (np.asarray(w1[l]) * g2).reshape(ET, 128, FF).transpose(1, 0, 2)
        w2_h[l] = np.asarray(w2[l]).reshape(UT, 128, E).transpose(1, 0, 2)
        vecs_h[l, 0:2 * HD, 0] = cq
        vecs_h[l, 0:2 * HD, 1] = ck
        vecs_h[l, :, 8:10] = bo_eff.reshape(2, 128).T
        vecs_h[l, :, 10:18] = b1_eff.reshape(8, 128).T
        vecs_h[l, :, 18:20] = np.asarray(b2[l]).astype(f32).reshape(2, 128).T

    w_head = np.asarray(w_head) * np.asarray(lnf_g).astype(f32)[:, None]
    vs = w_head.shape[1] // GROUP
    wh = np.zeros((E, vsp), f32)
    wh[:, :vs] = w_head[:, vs * j : vs * (j + 1)]
    whead_h = np.ascontiguousarray(wh.reshape(ET, 128, vsp).transpose(1, 0, 2))

    sp = np.arange(SC)[:, None]
    tp = np.arange(SC)[None, :]
    mask_h = (sp <= tp).astype(f32)

    nt = t // TB
    nsc = t // SC
    peye_h = np.zeros((128, nt, nt), f32)
    for tb in range(nt):
        peye_h[:, tb, tb] = 1.0
    vtc_h = np.ones((128, nsc, 2), f32)

    bf = NP_BF16
    return {
        "x0": np.ascontiguousarray(x0).astype(bf),
        "wqkv": np.ascontiguousarray(wqkv_h).astype(bf),
        "wo": np.ascontiguousarray(wo_h).astype(bf),
        "w1": np.ascontiguousarray(w1_h).astype(bf),
        "w2": np.ascontiguousarray(w2_h).astype(bf),
        "vecs": np.ascontiguousarray(vecs_h),
        "whead": whead_h.astype(bf),
        "mask": mask_h.astype(bf),
        "peye": peye_h.astype(bf),
        "vtc": vtc_h.astype(bf),
        "selp": np.ascontiguousarray(
            np.broadcast_to(np.eye(nt, dtype=f32)[:, :, None], (nt, nt, 128))
        ),
    }


_NC_CACHE = {}


def _get_nc():
    if "nc" not in _NC_CACHE:
        _NC_CACHE["nc"] = build_nc()
    return _NC_CACHE["nc"]


def kernel(**inputs):
    nc = _get_nc()
    in_maps = [prep_core_inputs(c, **inputs) for c in range(NCORES)]
    res = run_bass_kernel_spmd(nc, in_maps, list(range(NCORES)))
    out = np.empty((B, T, V), np.float32)
    for c in range(NCORES):
        b, j = c // GROUP, c % GROUP
        lg = res.results[c]["logits"]  # [128, T//128, VSP]
        lg = lg.transpose(1, 0, 2).reshape(T, VSP)
        out[b, :, VS * j : VS * (j + 1)] = lg[:, :VS].astype(np.float32)
    # b_head plus the final-LN bias folded through w_head (host-side)
    bh_eff = np.asarray(inputs["b_head"]).astype(np.float32) + (
        np.asarray(inputs["w_head"]).astype(np.float32).T
        @ np.asarray(inputs["lnf_b"]).astype(np.float32)
    )
    if np.any(bh_eff):
        out += bh_eff[None, None, :]
    return out


# revision 57
# speedup vs baseline: 1.3330x; 1.0029x over previous
"""Trainium2 Bass kernel for a 4-layer bigram-LM dense transformer.

Full-model shapes: B=2, T=2048, E=256, H=8, L=4, V=32000.

Sharding over 8 NeuronCores (self-contained, hardcoded):
  - 2-way data parallel over batch: cores 0-3 handle batch 0, cores 4-7
    batch 1 (a "batch group" of 4 cores each).
  - Within a batch group, per-token work (LN / QKV / wo / FFN) is
    replicated; attention (the exp-heavy part) is sharded 2 heads/core
    and re-assembled with one 4-rank AllGather per layer (bf16 payload).
  - lm_head is sharded 4-way over vocab columns within the group
    (8000 cols/core, padded to 8192), so the dominant logits write is
    split 8 ways and emitted as fp16 (host converts back to fp32).

Compute layout: activations live transposed [E, T] in SBUF so every
matmul contracts over the partition axis with zero transposes. All
activations and weights are bf16 (PSUM accumulation stays fp32), which
doubles DVE elementwise throughput and halves HBM/collective traffic.
LN gains are folded into the consuming weights host-side and LN biases
become per-output constants (q/k copy biases, bo/b1/b_head), so the LN
apply is just two bf16 tensor_tensor ops. Softmax skips the
max-subtraction (scores are ~1e-1 scale; exp cannot overflow) and row
sums ride along in the attention-output matmul via a ones column packed
next to V; the 1/rowsum is fanned across partitions with a K=1
ones-matmul (no DRAM round-trip). The two heads' score matmuls are
interleaved so they occupy different 32-row PE strips (tile_position)
and run concurrently; the two attn@V matmuls are packed into one PSUM
bank at column offsets 0 and 64 and also run concurrently. Logits are
emitted fp16 in a [128, T/128, V-shard] layout so eight token-chunks
batch into each DMA (the HWDGE fixed cost per descriptor-set is ~625ns),
and the lm_head matmul/copy pipeline rotates across four PSUM banks.
"""

import numpy as np
import ml_dtypes

import concourse.bass as bass
import concourse.mybir as mybir
import concourse.tile as tile
from concourse import bacc
from concourse.bass_utils import run_bass_kernel_spmd

AF = mybir.ActivationFunctionType
ALU = mybir.AluOpType
FP32 = mybir.dt.float32
FP32R = mybir.dt.float32r
BF16 = mybir.dt.bfloat16
FP16 = mybir.dt.float16
NP_BF16 = ml_dtypes.bfloat16

# model dims (full problem)
B, T, E, H, L, V = 2, 2048, 256, 8, 4, 32000
HD = E // H  # 32
EPS = 1e-5
NCORES = 8
GROUP = 4  # cores per batch group
HPC = H // GROUP  # heads per core (2)
VS = V // GROUP  # vocab shard per core (8000)
VSP = 8192  # padded vocab shard
TB = 512  # t-block (PSUM bank free dim)
SC = 128  # s-chunk (partition dim)
ET = E // 128  # embedding partition tiles (2)
FF = 4 * E  # 1024
UT = FF // 128  # ffn u-tiles (8)


def build_nc(t=T, layers=L, vsp=VSP, use_collective=True, ablate=()):
    """Build + compile the per-core Bass program (SPMD: same program, 8 cores)."""
    nt = t // TB      # t-blocks
    nsc = t // SC     # s-chunks
    ntc = t // 128    # t-chunks for lm head
    nvb = vsp // 512  # vocab blocks

    nc = bacc.Bacc("TRN2", num_devices=NCORES)

    # ---- DRAM parameters (per core) ----
    x0 = nc.declare_dram_parameter("x0", [E, t], BF16, isOutput=False)
    wqkv = nc.declare_dram_parameter("wqkv", [layers, 128, ET, 6 * HD], BF16, isOutput=False)
    wo_p = nc.declare_dram_parameter("wo", [layers, 128, ET, E], BF16, isOutput=False)
    w1_p = nc.declare_dram_parameter("w1", [layers, 128, ET, FF], BF16, isOutput=False)
    w2_p = nc.declare_dram_parameter("w2", [layers, 128, UT, E], BF16, isOutput=False)
    vecs = nc.declare_dram_parameter("vecs", [layers, 128, 20], FP32, isOutput=False)
    whead = nc.declare_dram_parameter("whead", [128, ET, vsp], BF16, isOutput=False)
    maskp = nc.declare_dram_parameter("mask", [SC, SC], BF16, isOutput=False)
    peye = nc.declare_dram_parameter("peye", [128, nt, nt], BF16, isOutput=False)
    vtc = nc.declare_dram_parameter("vtc", [128, nsc, 2], BF16, False)
    selp = nc.declare_dram_parameter("selp", [nt, nt, 128], FP32R, isOutput=False)
    logits = nc.declare_dram_parameter("logits", [128, t // 128, vsp], FP16, isOutput=True)

    # internal DRAM bounce buffers for the per-layer AllGather (bf16)
    cc_in = [nc.dram_tensor(f"cc_in{l}", [HPC * HD, t], BF16) for l in range(layers)]
    cc_out = [nc.dram_tensor(f"cc_out{l}", [GROUP * HPC * HD, t], BF16) for l in range(layers)]
    groups = [[0, 1, 2, 3], [4, 5, 6, 7]]

    from contextlib import ExitStack
    with tile.TileContext(nc) as tc:
        with ExitStack() as _ctx:
            persist = _ctx.enter_context(tc.tile_pool(name="persist", bufs=1))
            wpool2 = _ctx.enter_context(tc.tile_pool(name="wpool2", bufs=2))
            wpool1 = _ctx.enter_context(tc.tile_pool(name="wpool1", bufs=2))
            actp = _ctx.enter_context(tc.tile_pool(name="actp", bufs=1))
            xlnp = _ctx.enter_context(tc.tile_pool(name="xlnp", bufs=2))
            bigp = _ctx.enter_context(tc.tile_pool(name="bigp", bufs=3))
            expp = _ctx.enter_context(tc.tile_pool(name="expp", bufs=6))
            smallp = _ctx.enter_context(tc.tile_pool(name="smallp", bufs=3))
            tmpp = _ctx.enter_context(tc.tile_pool(name="tmpp", bufs=3))
            lgp = _ctx.enter_context(tc.tile_pool(name="lgp", bufs=3))
            whp = _ctx.enter_context(tc.tile_pool(name="whp", bufs=16))
            dpool = _ctx.enter_context(tc.tile_pool(name="dpool", bufs=2, space="DRAM"))
            ps_a = _ctx.enter_context(tc.tile_pool(name="ps_a", bufs=2, space="PSUM"))
            ps_o = _ctx.enter_context(tc.tile_pool(name="ps_o", bufs=2, space="PSUM"))
            ps_m = _ctx.enter_context(tc.tile_pool(name="ps_m", bufs=2, space="PSUM"))
            ps_s = _ctx.enter_context(tc.tile_pool(name="ps_s", bufs=1, space="PSUM"))
            # ---- persistent tiles ----
            xT = [persist.tile([128, t], BF16, tag=f"xT{e}", name=f"xT{e}") for e in range(ET)]
            for e in range(ET):
                nc.sync.dma_start(out=xT[e], in_=x0[128 * e : 128 * (e + 1), :])
            mask = persist.tile([SC, SC], BF16, tag="mask")
            nc.sync.dma_start(out=mask, in_=maskp[:, :])
            # v tile: per chunk cols = [vA(32) | ones | vB(32) | ones] so the
            # 33-wide per-head lhsT computes o rows 0:32 plus a row-sum row 32
            vt = persist.tile([128, nsc, 2, HD + 1], BF16, tag="vt")
            nc.sync.dma_start(out=vt[:, :, :, HD : HD + 1], in_=vtc[:, :, :])
            eyeblk = persist.tile([128, nt, nt], BF16, tag="eyeblk")
            nc.sync.dma_start(out=eyeblk, in_=peye[:, :, :])
            selt = persist.tile([nt, nt, 128], FP32R, tag="selt")
            nc.sync.dma_start(out=selt, in_=selp[:, :, :])
            # own heads' normalized attention out, pre-AllGather, [32, t] each
            oTp = [persist.tile([HD, t], BF16, tag=f"oTp{h}", name=f"oTp{h}") for h in range(HPC)]
            epst = persist.tile([128, 1], FP32, tag="epst")
            nc.vector.memset(epst, EPS)
            # ones rows at partitions 0 and 32: lhsT for the K=1 broadcast
            # matmul that fans the per-token 1/rowsum out to HD partitions
            ones33 = persist.tile([HD + 1, HD], FP32, tag="ones33")
            nc.vector.memset(ones33, 1.0)

            def layernorm(src, out_tiles):
                if "ln" in ablate:
                    for e in range(ET):
                        nc.scalar.activation(
                            out=out_tiles[e][:, :], in_=src[e][:, :], func=AF.Identity,
                        )
                    return
                """src: list of ET [128, t] bf16 tiles -> out_tiles bf16.

                Per-token stats via ones-matmuls into PSUM rows {0,32,64,96}
                (one per t-block), then x*s + m2 with s=rstd, m2=-mean*rstd
                broadcast along partitions. The LN gain/bias are folded into
                the consuming matmul's weights/biases host-side.
                """
                sq = [
                    bigp.tile([128, t], BF16, tag="big", name=f"sq{e}")
                    for e in range(ET)
                ]
                xs_ps = ps_s.tile([nt, TB], FP32, tag="stat_x")
                qs_ps = ps_s.tile([nt, TB], FP32, tag="stat_q")
                for tb in range(nt):
                    tbl = slice(TB * tb, TB * (tb + 1))
                    # split x^2 across DVE and ACT (ACT idles in LN phase;
                    # Square shares a table set with exp/ln fillers)
                    nc.vector.tensor_tensor(
                        out=sq[0][:, tbl], in0=src[0][:, tbl],
                        in1=src[0][:, tbl], op=ALU.mult,
                    )
                    nc.scalar.activation(
                        out=sq[1][:, tbl], in_=src[1][:, tbl], func=AF.Square,
                    )
                    for e in range(ET):
                        nc.tensor.matmul(
                            xs_ps[:, :],
                            eyeblk[:, tb, :],
                            src[e][:, tbl],
                            start=(tb == 0 and e == 0),
                            stop=(tb == nt - 1 and e == ET - 1),
                        )
                    for e in range(ET):
                        nc.tensor.matmul(
                            qs_ps[:, :],
                            eyeblk[:, tb, :],
                            sq[e][:, tbl],
                            start=(tb == 0 and e == 0),
                            stop=(tb == nt - 1 and e == ET - 1),
                        )
                mean4 = smallp.tile([nt, TB], FP32, tag="mean4", name="mean4")
                msq4 = smallp.tile([nt, TB], FP32, tag="msq4", name="msq4")
                var4 = smallp.tile([nt, TB], FP32, tag="var4", name="var4")
                s4 = smallp.tile([nt, TB], FP32R, tag="s4", name="s4")
                xs_rows = xs_ps[:, :]
                qs_rows = qs_ps[:, :]
                nc.vector.tensor_scalar(mean4[:, :], xs_rows, 1.0 / E, None, ALU.mult)
                nc.vector.tensor_scalar(msq4[:, :], qs_rows, 1.0 / E, None, ALU.mult)
                nc.vector.tensor_tensor(
                    out=var4[:, :], in0=mean4[:, :], in1=mean4[:, :], op=ALU.mult
                )
                nc.vector.tensor_tensor(
                    out=var4[:, :], in0=msq4[:, :], in1=var4[:, :], op=ALU.subtract
                )
                nc.scalar.activation(out=var4[:, :], in_=var4[:, :], func=AF.Ln, bias=epst[0:nt, :])
                nc.scalar.activation(out=s4[:, :], in_=var4[:, :], func=AF.Exp, scale=-0.5)
                m24 = smallp.tile([nt, TB], FP32R, tag="msq4", name="m24")
                nc.vector.scalar_tensor_tensor(
                    out=m24[:, :], in0=mean4[:, :], scalar=-1.0, in1=s4[:, :],
                    op0=ALU.mult, op1=ALU.mult,
                )
                for tb in range(nt):
                    s_bc = ps_s.tile([128, TB], FP32, tag="stat_x", name="s_bc")
                    m_bc = ps_s.tile([128, TB], FP32, tag="stat_q", name="m_bc")
                    nc.tensor.matmul(
                        s_bc[:, :], selt[:, tb, :], s4[:, :],
                        start=True, stop=True,
                    )
                    nc.tensor.matmul(
                        m_bc[:, :], selt[:, tb, :], m24[:, :],
                        start=True, stop=True,
                    )
                    # stage broadcasts to bf16 SBUF once per t-block so the
                    # per-e apply runs in the DVE 2x bf16 mode
                    s_sb = tmpp.tile([128, TB], BF16, tag="lntmp", name="s_sb")
                    m_sb = tmpp.tile([128, TB], BF16, tag="rbc", name="m_sb")
                    nc.scalar.copy(out=s_sb, in_=s_bc[:, :])
                    nc.scalar.copy(out=m_sb, in_=m_bc[:, :])
                    for e in range(ET):
                        tmp = bigp.tile([128, TB], BF16, tag="lnt2", name="lntmp2")
                        nc.vector.tensor_tensor(
                            out=tmp,
                            in0=src[e][:, TB * tb : TB * (tb + 1)],
                            in1=s_sb, op=ALU.mult,
                        )
                        nc.vector.tensor_tensor(
                            out=out_tiles[e][:, TB * tb : TB * (tb + 1)],
                            in0=tmp, in1=m_sb, op=ALU.add,
                        )

            # ================= layers =================
            for l in range(layers):
                wq_t = [wpool2.tile([128, 6 * HD], BF16, tag=f"wqkv{e}", name=f"wqkv{e}") for e in range(ET)]
                wo_t = [wpool2.tile([128, E], BF16, tag=f"wo{e}", name=f"wot{e}") for e in range(ET)]
                w1_t = [wpool1.tile([128, FF], BF16, tag=f"w1{e}", name=f"w1t{e}") for e in range(ET)]
                w2_t = wpool1.tile([128, UT, E], BF16, tag="w2")
                vec = wpool2.tile([128, 20], FP32, tag="vec")
                for e in range(ET):
                    nc.sync.dma_start(out=wq_t[e], in_=wqkv[l, :, e, :])
                    nc.sync.dma_start(out=wo_t[e], in_=wo_p[l, :, e, :])
                    nc.sync.dma_start(out=w1_t[e], in_=w1_p[l, :, e, :])
                nc.sync.dma_start(out=w2_t, in_=w2_p[l, :, :, :])
                nc.sync.dma_start(out=vec, in_=vecs[l, :, :])

                xln = [xlnp.tile([128, t], BF16, tag=f"xln{e}", name=f"xln{e}") for e in range(ET)]
                layernorm(xT, out_tiles=xln)

                qT = actp.tile([2 * HD, t], BF16, tag="qT")
                kT = actp.tile([2 * HD, t], BF16, tag="kT")
                for tb in range(nt):
                    tsl = slice(TB * tb, TB * (tb + 1))
                    qp = ps_m.tile([2 * HD, TB], FP32, tag="m")
                    for e in range(ET):
                        nc.tensor.matmul(
                            qp[:, :], wq_t[e][:, 0 : 2 * HD], xln[e][:, tsl],
                            start=(e == 0), stop=(e == ET - 1),
                        )
                    # +cq: the ln1_b contribution to q, folded host-side
                    nc.vector.tensor_scalar(
                        qT[:, tsl], qp[:, :], vec[0 : 2 * HD, 0:1], None, ALU.add
                    )
                    kp = ps_m.tile([2 * HD, TB], FP32, tag="m", name="kp")
                    for e in range(ET):
                        nc.tensor.matmul(
                            kp[:, :], wq_t[e][:, 2 * HD : 4 * HD], xln[e][:, tsl],
                            start=(e == 0), stop=(e == ET - 1),
                        )
                    nc.scalar.activation(
                        out=kT[:, tsl], in_=kp[:, :], func=AF.Identity,
                        bias=vec[0 : 2 * HD, 1:2],
                    )
                    for i in range(4 * tb, 4 * tb + 4):
                        vp = ps_o.tile([128, 2, HD], FP32, tag="o", name="vp")
                        for e in range(ET):
                            nc.tensor.matmul(
                                vp[:, :, :],
                                xln[e][:, SC * i : SC * (i + 1)],
                                wq_t[e][:, 4 * HD : 6 * HD],
                                start=(e == 0), stop=(e == ET - 1),
                            )
                        nc.any.tensor_copy(out=vt[:, i, :, 0:HD], in_=vp[:, :, :])

                    # ---- attention for this t-block (qkv ready up to here) ----
                    if "attn" in ablate:
                        if tb == 0:
                            for h in range(HPC):
                                nc.vector.memset(oTp[h].bitcast(FP16), 1.0)
                        continue
                    # both heads' o (+row-sum) packed in one PSUM bank:
                    # head h occupies partitions [64h, 64h+33)
                    op_ps = ps_o.tile([128, TB], FP32, tag="o", name="op_ps")
                    nmax = 4 * tb + 4

                    def emit_o(i, h, exh, d):
                        nc.tensor.matmul(
                            op_ps[64 * h : 64 * h + HD + 1, d:TB],
                            vt[:, i, h, :],
                            exh[:, d:TB],
                            start=(i == 0), stop=(i == nmax - 1),
                            tile_position=(0, 64 * h),
                        )

                    pend = []  # FIFO; AV trails exp by up to two chunks
                    for i in range(nmax):
                        d = max(0, SC * i - TB * tb)
                        psl = slice(d, TB)
                        tgl = slice(TB * tb + d, TB * (tb + 1))
                        cur = []
                        for h in range(HPC):
                            rsl = slice(32 * h, 32 * (h + 1))
                            at_ps = ps_a.tile([128, TB], FP32, tag="att", name=f"at_ps{h}")
                            exh = expp.tile([128, TB], BF16, tag=f"exp{h}", name="exh")
                            nc.tensor.matmul(
                                at_ps[:, psl],
                                kT[rsl, SC * i : SC * (i + 1)],
                                qT[rsl, tgl],
                                start=True, stop=True,
                                tile_position=(32 * h, 0),
                            )
                            nc.scalar.activation(
                                out=exh[:, psl], in_=at_ps[:, psl],
                                func=AF.Exp, scale=float(E) ** -0.5,
                            )
                            if i >= 4 * tb:  # diagonal chunk: mask upper triangle
                                nc.vector.tensor_tensor(
                                    out=exh[:, d : d + SC],
                                    in0=exh[:, d : d + SC],
                                    in1=mask[:, :], op=ALU.mult,
                                )
                            cur.append((i, h, exh, d))
                        pend.extend(cur)
                        while len(pend) > 2 * HPC:
                            emit_o(*pend.pop(0))
                    for ent in pend:
                        emit_o(*ent)
                    # normalize each head by its row-sum (psum row 64h+32):
                    # reciprocal -> K=1 ones-matmul broadcast across HD
                    # partitions (PSUM) -> multiply
                    sr = smallp.tile([HD + 1, TB], FP32, tag="srow", name="sr")
                    for h in range(HPC):
                        # DVE outputs must start at a 32-aligned partition
                        nc.vector.reciprocal(
                            out=sr[HD * h : HD * h + 1, :],
                            in_=op_ps[64 * h + HD : 64 * h + HD + 1, :],
                        )
                        rec_ps = ps_s.tile(
                            [HD, TB], FP32,
                            tag=("stat_x" if h == 0 else "stat_q"), name="rec_ps",
                        )
                        nc.tensor.matmul(
                            rec_ps[:, :],
                            ones33[HD * h : HD * h + 1, :],
                            sr[HD * h : HD * h + 1, :],
                            start=True, stop=True,
                        )
                        rec_sb = tmpp.tile([HD, TB], BF16, tag="rbc", name="rec_sb")
                        nc.vector.tensor_copy(out=rec_sb, in_=rec_ps[:, :])
                        nc.vector.tensor_tensor(
                            out=oTp[h][:, TB * tb : TB * (tb + 1)],
                            in0=op_ps[64 * h : 64 * h + HD, :],
                            in1=rec_sb,
                            op=ALU.mult,
                        )
                        # stream this t-block's slice to the AllGather bounce
                        # buffer now, so the collective input is ready the
                        # moment the last block finishes
                        nc.sync.dma_start(
                            out=cc_in[l][HD * h : HD * (h + 1), TB * tb : TB * (tb + 1)],
                            in_=oTp[h][:, TB * tb : TB * (tb + 1)],
                        )

                # ---- AllGather heads across the 4-core batch group ----
                oT = [actp.tile([128, t], BF16, tag=tg, name=f"oT_{tg}") for tg in ("qT", "kT")]
                if use_collective:
                    nc.gpsimd.collective_compute(
                        "AllGather", ALU.bypass,
                        replica_groups=groups,
                        ins=[cc_in[l][:, :]], outs=[cc_out[l][:, :]],
                    )
                    for e in range(ET):
                        nc.sync.dma_start(
                            out=oT[e], in_=cc_out[l][128 * e : 128 * (e + 1), :]
                        )
                else:
                    # no-comm build (used for TimelineSim): same bounce DMAs as
                    # the real path so DMA time is modeled; cc_out carries
                    # no meaningful data (timing-only build)
                    for e in range(ET):
                        nc.sync.dma_start(
                            out=oT[e], in_=cc_out[l][128 * e : 128 * (e + 1), :]
                        )

                # ---- wo projection + residual ----
                for tb in range(nt):
                    tsl = slice(TB * tb, TB * (tb + 1))
                    for eo in range(ET):
                        wpool = ps_m if eo % 2 == 0 else ps_a
                        wp = wpool.tile([128, TB], FP32, tag="m" if eo % 2 == 0 else "att")
                        for e in range(ET):
                            nc.tensor.matmul(
                                wp[:, :],
                                wo_t[e][:, 128 * eo : 128 * (eo + 1)],
                                oT[e][:, tsl],
                                start=(e == 0), stop=(e == ET - 1),
                            )
                        nc.vector.scalar_tensor_tensor(
                            out=xT[eo][:, tsl], in0=wp[:, :],
                            scalar=vec[:, 8 + eo : 9 + eo], in1=xT[eo][:, tsl],
                            op0=ALU.add, op1=ALU.add,
                        )

                # ---- FFN ----
                xln2 = [xlnp.tile([128, t], BF16, tag=f"xln{e}", name=f"xln{e}") for e in range(ET)]
                layernorm(xT, out_tiles=xln2)
                for tb in range(nt):
                    if "ffn" in ablate:
                        break
                    tsl = slice(TB * tb, TB * (tb + 1))
                    ru_halves = []
                    for half in range(2):
                        ru = bigp.tile([128, UT // 2, TB], BF16, tag="big", name="ru")
                        for uu in range(UT // 2):
                            ut = half * (UT // 2) + uu
                            upool = ps_a if uu % 2 == 0 else ps_o
                            up = upool.tile([128, TB], FP32, tag="att" if uu % 2 == 0 else "o", name="up")
                            for e in range(ET):
                                nc.tensor.matmul(
                                    up[:, :],
                                    w1_t[e][:, 128 * ut : 128 * (ut + 1)],
                                    xln2[e][:, tsl],
                                    start=(e == 0), stop=(e == ET - 1),
                                )
                            nc.scalar.activation(
                                out=ru[:, uu, :], in_=up[:, :], func=AF.Relu,
                                bias=vec[:, 10 + ut : 11 + ut],
                            )
                        ru_halves.append(ru)
                    for eo in range(ET):
                        wp2 = ps_m.tile([128, TB], FP32, tag="m", name="wp2")
                        for ut in range(UT):
                            nc.tensor.matmul(
                                wp2[:, :],
                                w2_t[:, ut, 128 * eo : 128 * (eo + 1)],
                                ru_halves[ut // (UT // 2)][:, ut % (UT // 2), :],
                                start=(ut == 0), stop=(ut == UT - 1),
                            )
                        nc.vector.scalar_tensor_tensor(
                            out=xT[eo][:, tsl], in0=wp2[:, :],
                            scalar=vec[:, 18 + eo : 19 + eo], in1=xT[eo][:, tsl],
                            op0=ALU.add, op1=ALU.add,
                        )

            # ================= final LN + lm_head =================
            xf = [xlnp.tile([128, t], BF16, tag=f"xln{e}", name=f"xln{e}") for e in range(ET)]
            layernorm(xT, out_tiles=xf)
            GB = min(8, ntc)  # token-chunks batched per logits DMA
            # token-group-major: all vocab blocks for the first GB token
            # chunks run as soon as the first half of the final LN lands.
            # Weight tiles are loaded once (g==0) and kept resident.
            nvb_r = nvb if "lm" not in ablate else 1
            whs = []
            for g in range(ntc // GB):
                for vb in range(nvb_r):
                    if g == 0:
                        wh = whp.tile([128, ET, 512], BF16, tag="wh", name=f"wh{vb}")
                        nc.sync.dma_start(out=wh, in_=whead[:, :, 512 * vb : 512 * (vb + 1)])
                        whs.append(wh)
                    else:
                        wh = whs[vb]
                    lg = lgp.tile([128, GB, 512], FP16, tag="lg")
                    for k in range(GB):
                        tcn = GB * g + k
                        # rotate across three PSUM pools (6 banks) so the
                        # matmul/copy pipeline never waits on a bank
                        lpool = (ps_m, ps_a, ps_o)[k % 3]
                        lp = lpool.tile([128, 512], FP32, tag=("m", "att", "o")[k % 3])
                        for e in range(ET):
                            nc.tensor.matmul(
                                lp[:, :],
                                xf[e][:, 128 * tcn : 128 * (tcn + 1)],
                                wh[:, e, :],
                                start=(e == 0), stop=(e == ET - 1),
                            )
                        if (vb + tcn) % 2 == 0:
                            nc.vector.tensor_copy(out=lg[:, k, :], in_=lp[:, :])
                        else:
                            nc.scalar.copy(out=lg[:, k, :], in_=lp[:, :])
                    nc.sync.dma_start(
                        out=logits[:, GB * g : GB * (g + 1), 512 * vb : 512 * (vb + 1)],
                        in_=lg,
                    )

    nc.compile()
    return nc


# ---------------- host-side prep / unshard ----------------

def prep_core_inputs(c, X, tok_emb, pos_emb, wq, wk, wv, wo, bo, w1, b1, w2, b2,
                     ln1_g, ln1_b, ln2_g, ln2_b, lnf_g, lnf_b, w_head, b_head,
                     t=T, layers=L, vsp=VSP):
    b = c // GROUP
    j = c % GROUP
    heads = [HPC * j + k for k in range(HPC)]

    f32 = np.float32
    Xb = np.asarray(X[b]).astype(np.int64)
    x0 = (np.asarray(tok_emb)[Xb] + np.asarray(pos_emb)[:t]).astype(f32).T  # [E, t]

    wq = np.asarray(wq); wk = np.asarray(wk); wv = np.asarray(wv)
    wqkv_h = np.empty((layers, 128, ET, 6 * HD), f32)
    wo_h = np.empty((layers, 128, ET, E), f32)
    w1_h = np.empty((layers, 128, ET, FF), f32)
    w2_h = np.empty((layers, 128, UT, E), f32)
    vecs_h = np.zeros((layers, 128, 20), f32)
    for l in range(layers):
        # fold LN gains into the consuming weights and LN biases into
        # per-output-constant corrections (exact for affine LN):
        #   xln_true = xln_raw * g + b  =>  W^T xln_true = (gW)^T xln_raw + W^T b
        g1 = np.asarray(ln1_g[l]).astype(f32)[:, None]
        b1n = np.asarray(ln1_b[l]).astype(f32)
        g2 = np.asarray(ln2_g[l]).astype(f32)[:, None]
        b2n = np.asarray(ln2_b[l]).astype(f32)
        qc = np.concatenate([wq[l, h] for h in heads], axis=1)  # [E, 64]
        kc = np.concatenate([wk[l, h] for h in heads], axis=1)
        vc = np.concatenate([wv[l, h] for h in heads], axis=1)
        cq = qc.T @ b1n  # [64] q bias from ln1_b
        ck = kc.T @ b1n
        # v bias from ln1_b for ALL heads, folded through wo into bo
        cv_full = np.concatenate([wv[l, h].T @ b1n for h in range(H)])  # [E]
        bo_eff = np.asarray(bo[l]).astype(f32) + np.asarray(wo[l]).T @ cv_full
        b1_eff = np.asarray(b1[l]).astype(f32) + np.asarray(w1[l]).T @ b2n
        qkv = np.concatenate([qc, kc, vc], axis=1) * g1  # [E, 192]
        wqkv_h[l] = qkv.reshape(ET, 128, 6 * HD).transpose(1, 0, 2)
        wo_h[l] = np.asarray(wo[l]).reshape(ET, 128, E).transpose(1, 0, 2)
        w1_h[l] = (np.asarray(w1[l]) * g2).reshape(ET, 128, FF).transpose(1, 0, 2)
        w2_h[l] = np.asarray(w2[l]).reshape(UT, 128, E).transpose(1, 0, 2)
        vecs_h[l, 0:2 * HD, 0] = cq
        vecs_h[l, 0:2 * HD, 1] = ck
        vecs_h[l, :, 8:10] = bo_eff.reshape(2, 128).T
        vecs_h[l, :, 10:18] = b1_eff.reshape(8, 128).T
        vecs_h[l, :, 18:20] = np.asarray(b2[l]).astype(f32).reshape(2, 128).T

    w_head = np.asarray(w_head) * np.asarray(lnf_g).astype(f32)[:, None]
    vs = w_head.shape[1] // GROUP
    wh = np.zeros((E, vsp), f32)
    wh[:, :vs] = w_head[:, vs * j : vs * (j + 1)]
    whead_h = np.ascontiguousarray(wh.reshape(ET, 128, vsp).transpose(1, 0, 2))

    sp = np.arange(SC)[:, None]
    tp = np.arange(SC)[None, :]
    mask_h = (sp <= tp).astype(f32)

    nt = t // TB
    nsc = t // SC
    peye_h = np.zeros((128, nt, nt), f32)
    for tb in range(nt):
        peye_h[:, tb, tb] = 1.0
    vtc_h = np.ones((128, nsc, 2), f32)

    bf = NP_BF16
    return {
        "x0": np.ascontiguousarray(x0).astype(bf),
        "wqkv": np.ascontiguousarray(wqkv_h).astype(bf),
        "wo": np.ascontiguousarray(wo_h).astype(bf),
        "w1": np.ascontiguousarray(w1_h).astype(bf),
        "w2": np.ascontiguousarray(w2_h).astype(bf),
        "vecs": np.ascontiguousarray(vecs_h),
        "whead": whead_h.astype(bf),
        "mask": mask_h.astype(bf),
        "peye": peye_h.astype(bf),
        "vtc": vtc_h.astype(bf),
        "selp": np.ascontiguousarray(
            np.broadcast_to(np.eye(nt, dtype=f32)[:, :, None], (nt, nt, 128))
        ),
    }


_NC_CACHE = {}


def _get_nc():
    if "nc" not in _NC_CACHE:
        _NC_CACHE["nc"] = build_nc()
    return _NC_CACHE["nc"]


def kernel(**inputs):
    nc = _get_nc()
    in_maps = [prep_core_inputs(c, **inputs) for c in range(NCORES)]
    res = run_bass_kernel_spmd(nc, in_maps, list(range(NCORES)))
    out = np.empty((B, T, V), np.float32)
    for c in range(NCORES):
        b, j = c // GROUP, c % GROUP
        lg = res.results[c]["logits"]  # [128, T//128, VSP]
        lg = lg.transpose(1, 0, 2).reshape(T, VSP)
        out[b, :, VS * j : VS * (j + 1)] = lg[:, :VS].astype(np.float32)
    # b_head plus the final-LN bias folded through w_head (host-side)
    bh_eff = np.asarray(inputs["b_head"]).astype(np.float32) + (
        np.asarray(inputs["w_head"]).astype(np.float32).T
        @ np.asarray(inputs["lnf_b"]).astype(np.float32)
    )
    if np.any(bh_eff):
        out += bh_eff[None, None, :]
    return out
